# revision 8
# baseline (speedup 1.0000x reference)
"""MoE MLP (top-2 of 8 experts) Trainium2 kernel.

Strategy: expert-parallel across the 8 NeuronCores (host computes the tiny
top-2 gating exactly in fp32 and gathers each expert's tokens into a
capacity-padded buffer; core e runs expert e's two big matmuls).

The matmuls run as fp8(e4m3) DoubleRow matmuls with a 3-term residual
decomposition. Each operand is split into an fp8 base plus an fp8 residual
(quantization error of the base), and the product keeps the three large
cross-terms:

    x @ W  ~=  x8@W8 + x8@Wr8 + xr8@W8        (drop xr8@Wr8, ~2^-8 relative)

DoubleRow pairs two 128-row contraction subtiles per PE instruction at half
the per-column cost, so the 3-term product costs 0.75x of one bf16 matmul
while keeping ~2^-8 effective precision (measured end-to-end rel err ~2e-3).

Weights are pre-scaled by 32 before fp8 quantization so their ~0.02-scale
entries sit in e4m3's normal range; the 1/32 descale folds into the mm1
activation scale and the per-token gate (g/32) on mm2 eviction.

Device layout keeps the contraction dim on SBUF partitions throughout:
  mm1: psum[h_blk(128), tok] += DR-pairs of {wfc8,wfcr8}[d,h_blk] x {x8,xr8}[d,tok]
  act: t = prelu(psum/32, 0.5) (fp16); af = t*t (fp16);
       a8 = fp8(af); ar8 = fp8(af - a8)
  mm2: psum[tok(128), d(512)] += DR-pairs of {a8,ar8}[h,tok] x {wp8,wpr8}[h,d]
  evict: out = psum * (g[token]/32)

Host scatters per-expert rows back (each token appears in exactly 2 expert
lists) and sums - identical math to the reference's dense masked combine.
"""

import numpy as np
import ml_dtypes
from contextlib import ExitStack

B, T, D, H, E = 4, 2048, 1024, 4096, 8
N = B * T
P = 128
CHUNK = 512
SW = 32.0  # weight pre-scale before fp8 quantization

E4NP = ml_dtypes.float8_e4m3  # == concourse mybir.dt.float8e4


_NC_CACHE = {}


def _build_nc(C):
    """Build + compile the per-core Bass program for capacity C tokens.

    C must be a multiple of 128. Tokens stream in chunks of 512 plus one
    optional tail chunk of C % 512. The program depends only on C, so it is
    cached: reusing the same nc object also lets bass2jax's jit cache skip
    the NEFF compile on repeat kernel() calls.
    """
    if C in _NC_CACHE:
        return _NC_CACHE[C]
    import concourse.bacc as bacc
    import concourse.tile as tile
    import concourse.mybir as mybir

    assert C % P == 0
    f8 = mybir.dt.float8e4
    f16 = mybir.dt.float16
    f32 = mybir.dt.float32
    AF = mybir.ActivationFunctionType
    DR = mybir.MatmulPerfMode.DoubleRow
    MUL = mybir.AluOpType.mult
    SUB = mybir.AluOpType.subtract

    nc = bacc.Bacc(None, target_bir_lowering=False, debug=False)
    x8T = nc.dram_tensor("x8T", [D, C], f8, kind="ExternalInput")
    xr8T = nc.dram_tensor("xr8T", [D, C], f8, kind="ExternalInput")
    wfc8T = nc.dram_tensor("wfc8T", [D, H], f8, kind="ExternalInput")
    wfcr8T = nc.dram_tensor("wfcr8T", [D, H], f8, kind="ExternalInput")
    wp8T = nc.dram_tensor("wp8T", [H, D], f8, kind="ExternalInput")
    wpr8T = nc.dram_tensor("wpr8T", [H, D], f8, kind="ExternalInput")
    g = nc.dram_tensor("g", [P, C // P], f32, kind="ExternalInput")
    out = nc.dram_tensor("outp", [C, D], f32, kind="ExternalOutput")

    x8T_v = x8T.ap().rearrange("(ko p) c -> p ko c", p=P)        # [128, 8, C]
    xr8T_v = xr8T.ap().rearrange("(ko p) c -> p ko c", p=P)
    wfc8_v = wfc8T.ap().rearrange("(ko p) h -> p ko h", p=P)     # [128, 8, H]
    wfcr8_v = wfcr8T.ap().rearrange("(ko p) h -> p ko h", p=P)
    wp8_v = wp8T.ap().rearrange("(ko p) d -> p ko d", p=P)       # [128, 32, D]
    wpr8_v = wpr8T.ap().rearrange("(ko p) d -> p ko d", p=P)
    out_v = out.ap().rearrange("(c p) d -> p c d", p=P)          # [128, C//128, D]

    KD = D // P          # 8  k-subtiles for mm1
    KH = H // P          # 32 k-subtiles for mm2 (and h-blocks of mm1 output)
    DN = D // CHUNK      # 2 output-column blocks

    chunks = [CHUNK] * (C // CHUNK)
    if C % CHUNK:
        chunks.append(C % CHUNK)

    with tile.TileContext(nc) as tc:
        with ExitStack() as ctx:
            const = ctx.enter_context(tc.tile_pool(name="const", bufs=1))
            xpool = ctx.enter_context(tc.tile_pool(name="xp", bufs=2))
            apool = ctx.enter_context(tc.tile_pool(name="apool", bufs=1))
            tpool = ctx.enter_context(tc.tile_pool(name="tp", bufs=2))
            opool = ctx.enter_context(tc.tile_pool(name="op", bufs=4))
            ps1pool = ctx.enter_context(tc.tile_pool(name="ps1", bufs=4, space="PSUM"))
            ps2pool = ctx.enter_context(tc.tile_pool(name="ps2", bufs=3, space="PSUM"))
            warmpool = ctx.enter_context(tc.tile_pool(name="wm", bufs=1, space="PSUM"))

            # Startup-critical DMAs first. DGE descriptor generation is a
            # serialized per-path resource, so use few, large transfers and
            # spread them over all three DGE paths: SP (sync), Activation
            # (scalar) and SWDGE (gpsimd). mm1 h-block mh of chunk 0 reads
            # (in order) x8+wfc8 cols of mh, xr8, then wfcr8 cols of mh.
            x8_tiles = {}
            xr8_tiles = {}
            x8_tiles[0] = xpool.tile([P, KD, chunks[0]], f8, tag="x8", name="x8t0")
            xr8_tiles[0] = xpool.tile([P, KD, chunks[0]], f8, tag="xr8", name="xr8t0")
            wfc8_sb = const.tile([P, KD, H], f8)
            wfcr8_sb = const.tile([P, KD, H], f8)
            nc.sync.dma_start(wfc8_sb[:, :, 0:P], wfc8_v[:, :, 0:P])
            nc.scalar.dma_start(wfcr8_sb[:, :, 0:P], wfcr8_v[:, :, 0:P])
            nc.sync.dma_start(x8_tiles[0][:, 0:4, :], x8T_v[:, 0:4, 0:chunks[0]])
            nc.scalar.dma_start(xr8_tiles[0][:, 0:4, :], xr8T_v[:, 0:4, 0:chunks[0]])
            nc.sync.dma_start(x8_tiles[0][:, 4:KD, :], x8T_v[:, 4:KD, 0:chunks[0]])
            nc.scalar.dma_start(xr8_tiles[0][:, 4:KD, :], xr8T_v[:, 4:KD, 0:chunks[0]])
            # Rest of wfc8/wfcr8 in H-slices, finer early (tighter deadline),
            # coarser later: mm1 h-block mh only depends on the slices
            # covering its 128 cols (subregion deps).
            bounds = [P, 2 * P, 4 * P, 6 * P, 8 * P, 12 * P, 16 * P,
                      20 * P, 24 * P, 28 * P, H]
            for s0, s1 in zip(bounds[:-1], bounds[1:]):
                nc.sync.dma_start(
                    wfc8_sb[:, :, s0:s1], wfc8_v[:, :, s0:s1]
                )
                nc.scalar.dma_start(
                    wfcr8_sb[:, :, s0:s1], wfcr8_v[:, :, s0:s1]
                )
            # wproj is needed once mm2 of chunk 0 starts (~41us in). It goes
            # on the software DGE (Pool engine) so descriptor generation runs
            # in parallel with the wfc/x loads, but gated behind a dummy
            # dependency on an early a8 block so its transfers don't steal
            # DMA bandwidth from the startup-critical wfc stream (issued in
            # the chunk-0 loop below).
            wp8_sb = const.tile([P, KH, D], f8)
            wpr8_sb = const.tile([P, KH, D], f8)
            g_sb = const.tile([P, C // P], f32)
            dep_sb = const.tile([P, 1], f8)

            def load_wproj():
                for dn0 in range(DN):
                    dc = slice(dn0 * CHUNK, (dn0 + 1) * CHUNK)
                    for kc in range(2):
                        sl = slice(kc * (KH // 2), (kc + 1) * (KH // 2))
                        nc.gpsimd.dma_start(wp8_sb[:, sl, dc], wp8_v[:, sl, dc])
                    for kc in range(2):
                        sl = slice(kc * (KH // 2), (kc + 1) * (KH // 2))
                        nc.gpsimd.dma_start(wpr8_sb[:, sl, dc], wpr8_v[:, sl, dc])
                nc.gpsimd.dma_start(g_sb[:], g.ap())

            # PE warmup: the HAM clock-gate needs ~3.4us of sustained matmul
            # activity to grant the 2.4 GHz rate. The PE is idle waiting for
            # the first DMAs anyway, so burn that window on dummy matmuls
            # over a zeroed scratch tile (results never read).
            warm_sb = const.tile([P, P], f16)
            nc.vector.memset(warm_sb[:], 0.0)
            warm_ps = warmpool.tile([P, P], f32)
            for _ in range(38):
                nc.tensor.matmul(warm_ps[:], warm_sb[:], warm_sb[:],
                                 start=True, stop=True)

            tok0 = 0
            for c, S in enumerate(chunks):
                x8_t = x8_tiles[c]
                xr8_t = xr8_tiles[c]
                a8_t = apool.tile([P, KH, S], f8, tag="a8")
                ar8_t = apool.tile([P, KH, S], f8, tag="ar8")
                for mh in range(KH):
                    ps1 = ps1pool.tile([P, S], f32, tag="ps1")
                    mcols = slice(mh * P, (mh + 1) * P)
                    for k in range(0, KD, 2):
                        nc.tensor.matmul(
                            ps1[:], wfc8_sb[:, k:k + 2, mcols],
                            x8_t[:, k:k + 2, :],
                            start=(k == 0), stop=False, perf_mode=DR,
                        )
                    for k in range(0, KD, 2):
                        nc.tensor.matmul(
                            ps1[:], wfc8_sb[:, k:k + 2, mcols],
                            xr8_t[:, k:k + 2, :],
                            start=False, stop=False, perf_mode=DR,
                        )
                    for k in range(0, KD, 2):
                        nc.tensor.matmul(
                            ps1[:], wfcr8_sb[:, k:k + 2, mcols],
                            x8_t[:, k:k + 2, :],
                            start=False, stop=(k == KD - 2), perf_mode=DR,
                        )
                    # t = prelu(psum/32, 0.5); af = t*t; a8 = fp8(af);
                    # ar8 = fp8(af - a8)
                    t_t = tpool.tile([P, S], f16, tag="t")
                    nc.scalar.activation(t_t[:], ps1[:], AF.Prelu,
                                         alpha=0.5, scale=1.0 / SW)
                    af_t = tpool.tile([P, S], f16, tag="af")
                    nc.vector.tensor_tensor(af_t[:], t_t[:], t_t[:], MUL)
                    nc.scalar.activation(a8_t[:, mh, :], af_t[:], AF.Copy)
                    nc.vector.tensor_tensor(
                        ar8_t[:, mh, :], af_t[:], a8_t[:, mh, :], SUB
                    )
                    if c == 0 and mh == 6:
                        # Dummy dep: gates the Pool-engine wproj loads on
                        # mm1 progress (~15us in) so they don't contend
                        # with the wfc stream at startup.
                        nc.gpsimd.tensor_copy(dep_sb[:], a8_t[:, 6, 0:1])
                        load_wproj()
                # Prefetch next chunk's tokens now, ahead of this chunk's
                # out-DMAs in the DGE queues.
                if c + 1 < len(chunks):
                    S1 = chunks[c + 1]
                    t1 = tok0 + S
                    x8_tiles[c + 1] = xpool.tile(
                        [P, KD, S1], f8, tag="x8", name=f"x8t{c + 1}"
                    )
                    xr8_tiles[c + 1] = xpool.tile(
                        [P, KD, S1], f8, tag="xr8", name=f"xr8t{c + 1}"
                    )
                    nc.sync.dma_start(x8_tiles[c + 1][:], x8T_v[:, :, t1:t1 + S1])
                    nc.scalar.dma_start(xr8_tiles[c + 1][:], xr8T_v[:, :, t1:t1 + S1])
                for ti in range(S // P):
                    gcol = tok0 // P + ti
                    tcols = slice(ti * P, (ti + 1) * P)
                    for dn in range(DN):
                        dcols = slice(dn * CHUNK, (dn + 1) * CHUNK)
                        ps2 = ps2pool.tile([P, CHUNK], f32, tag="ps2")
                        for k in range(0, KH, 2):
                            nc.tensor.matmul(
                                ps2[:], a8_t[:, k:k + 2, tcols],
                                wp8_sb[:, k:k + 2, dcols],
                                start=(k == 0), stop=False, perf_mode=DR,
                            )
                        for k in range(0, KH, 2):
                            nc.tensor.matmul(
                                ps2[:], ar8_t[:, k:k + 2, tcols],
                                wp8_sb[:, k:k + 2, dcols],
                                start=False, stop=False, perf_mode=DR,
                            )
                        for k in range(0, KH, 2):
                            nc.tensor.matmul(
                                ps2[:], a8_t[:, k:k + 2, tcols],
                                wpr8_sb[:, k:k + 2, dcols],
                                start=False, stop=(k == KH - 2), perf_mode=DR,
                            )
                        o_tile = opool.tile([P, CHUNK], f32, tag="ot")
                        # fused gate+descale: out = psum * (g[token]/32)
                        nc.scalar.activation(
                            o_tile[:], ps2[:], AF.Copy,
                            scale=g_sb[:, gcol:gcol + 1],
                        )
                        nc.sync.dma_start(
                            out_v[:, gcol, dcols], o_tile[:]
                        )
                tok0 += S
    nc.compile()
    _NC_CACHE[C] = nc
    return nc


def _route(xf, Wg):
    """Exact top-2 gating in fp32, mirroring the reference math."""
    logits = xf @ Wg.T                                   # [N, E]
    top2 = np.argpartition(logits, E - 2, axis=1)[:, E - 2:]   # [N, 2] unordered
    vals = np.take_along_axis(logits, top2, axis=1)
    m = vals.max(axis=1, keepdims=True)
    ex = np.exp(vals - m)
    w = ex / ex.sum(axis=1, keepdims=True)               # [N, 2] softmax over top-2
    return top2, w


def _q8(v):
    """fp8(e4m3) base + residual split of a fp32 array."""
    base = v.astype(E4NP)
    resid = (v - base.astype(np.float32)).astype(E4NP)
    return base, resid


def run_moe(x, Wg, Wfc, Wproj, trace=False):
    from concourse import bass_utils

    xf = np.ascontiguousarray(x.reshape(-1, D), dtype=np.float32)
    top2, w = _route(xf, Wg.astype(np.float32))

    toks, gates = [], []
    for e in range(E):
        sel = np.nonzero((top2 == e).any(axis=1))[0]
        ge = (w[sel] * (top2[sel] == e)).sum(axis=1).astype(np.float32)
        toks.append(sel)
        gates.append(ge)

    maxc = max(len(t) for t in toks)
    C = max(P, ((maxc + P - 1) // P) * P)

    nc = _build_nc(C)

    x8_full, xr8_full = _q8(xf)
    in_maps = []
    for e in range(E):
        te = toks[e]
        x8T_e = np.zeros((D, C), E4NP)
        x8T_e[:, :len(te)] = x8_full[te].T
        xr8T_e = np.zeros((D, C), E4NP)
        xr8T_e[:, :len(te)] = xr8_full[te].T
        wfc8, wfcr8 = _q8(np.ascontiguousarray(Wfc[e].T, np.float32) * SW)
        wp8, wpr8 = _q8(np.ascontiguousarray(Wproj[e].T, np.float32) * SW)
        g_e = np.zeros((C,), np.float32)
        g_e[:len(te)] = gates[e] / SW
        g_mat = np.ascontiguousarray(g_e.reshape(C // P, P).T)
        in_maps.append({
            "x8T": x8T_e,
            "xr8T": xr8T_e,
            "wfc8T": wfc8,
            "wfcr8T": wfcr8,
            "wp8T": wp8,
            "wpr8T": wpr8,
            "g": g_mat,
        })

    # NTFF tracing is unavailable under this axon environment (no
    # antenv.axon_hooks); always run untraced.
    res = bass_utils.run_bass_kernel_spmd(
        nc, in_maps, core_ids=list(range(E)), trace=False
    )

    out = np.zeros((N, D), np.float32)
    for e in range(E):
        te = toks[e]
        out[te] += res.results[e]["outp"][:len(te)]
    return out.reshape(B, T, D), res


def kernel(x, Wg, Wfc, Wproj):
    out, _ = run_moe(np.asarray(x), np.asarray(Wg), np.asarray(Wfc), np.asarray(Wproj))
    return out


# revision 10
# speedup vs baseline: 1.0566x; 1.0566x over previous
"""MoE MLP (top-2 of 8 experts) Trainium2 kernel.

Strategy: expert-parallel across the 8 NeuronCores (host computes the tiny
top-2 gating exactly in fp32 and gathers each expert's tokens into a
capacity-padded buffer; core e runs expert e's two big matmuls).

The matmuls run as fp8(e4m3) DoubleRow matmuls with a 3-term residual
decomposition. Each operand is split into an fp8 base plus an fp8 residual
(quantization error of the base), and the product keeps the three large
cross-terms:

    x @ W  ~=  x8@W8 + x8@Wr8 + xr8@W8        (drop xr8@Wr8, ~2^-8 relative)

DoubleRow pairs two 128-row contraction subtiles per PE instruction at half
the per-column cost, so the 3-term product costs 0.75x of one bf16 matmul
while keeping ~2^-8 effective precision (measured end-to-end rel err ~2e-3).

Weights are pre-scaled by 32 before fp8 quantization so their ~0.02-scale
entries sit in e4m3's normal range; the 1/32 descale folds into the mm1
activation scale and the per-token gate (g/32) on mm2 eviction.

Device layout keeps the contraction dim on SBUF partitions throughout:
  mm1: psum[h_blk(128), tok] += DR-pairs of {wfc8,wfcr8}[d,h_blk] x {x8,xr8}[d,tok]
  act: t = prelu(psum/32, 0.5) (fp16); af = t*t (fp16);
       a8 = fp8(af); ar8 = fp8(af - a8)
  mm2: psum[tok(128), d(512)] += DR-pairs of {a8,ar8}[h,tok] x {wp8,wpr8}[h,d]
  evict: out = psum * (g[token]/32)

Host scatters per-expert rows back (each token appears in exactly 2 expert
lists) and sums - identical math to the reference's dense masked combine.
"""

import numpy as np
import ml_dtypes
from contextlib import ExitStack

B, T, D, H, E = 4, 2048, 1024, 4096, 8
N = B * T
P = 128
CHUNK = 512
SW = 32.0  # weight pre-scale before fp8 quantization

E4NP = ml_dtypes.float8_e4m3  # == concourse mybir.dt.float8e4


_NC_CACHE = {}


def _build_nc(C):
    """Build + compile the per-core Bass program for capacity C tokens.

    C must be a multiple of 128. Tokens stream in chunks of 512 plus one
    optional tail chunk of C % 512. The program depends only on C, so it is
    cached: reusing the same nc object also lets bass2jax's jit cache skip
    the NEFF compile on repeat kernel() calls.
    """
    if C in _NC_CACHE:
        return _NC_CACHE[C]
    import concourse.bacc as bacc
    import concourse.tile as tile
    import concourse.mybir as mybir

    assert C % P == 0
    f8 = mybir.dt.float8e4
    f16 = mybir.dt.float16
    f32 = mybir.dt.float32
    AF = mybir.ActivationFunctionType
    DR = mybir.MatmulPerfMode.DoubleRow
    MUL = mybir.AluOpType.mult
    SUB = mybir.AluOpType.subtract

    nc = bacc.Bacc(None, target_bir_lowering=False, debug=False)
    x8T = nc.dram_tensor("x8T", [D, C], f8, kind="ExternalInput")
    xr8T = nc.dram_tensor("xr8T", [D, C], f8, kind="ExternalInput")
    wfc8T = nc.dram_tensor("wfc8T", [D, H], f8, kind="ExternalInput")
    wfcr8T = nc.dram_tensor("wfcr8T", [D, H], f8, kind="ExternalInput")
    wp8T = nc.dram_tensor("wp8T", [H, D], f8, kind="ExternalInput")
    wpr8T = nc.dram_tensor("wpr8T", [H, D], f8, kind="ExternalInput")
    g = nc.dram_tensor("g", [P, C // P], f32, kind="ExternalInput")
    out = nc.dram_tensor("outp", [C, D], f32, kind="ExternalOutput")

    x8T_v = x8T.ap().rearrange("(ko p) c -> p ko c", p=P)        # [128, 8, C]
    xr8T_v = xr8T.ap().rearrange("(ko p) c -> p ko c", p=P)
    wfc8_v = wfc8T.ap().rearrange("(ko p) h -> p ko h", p=P)     # [128, 8, H]
    wfcr8_v = wfcr8T.ap().rearrange("(ko p) h -> p ko h", p=P)
    wp8_v = wp8T.ap().rearrange("(ko p) d -> p ko d", p=P)       # [128, 32, D]
    wpr8_v = wpr8T.ap().rearrange("(ko p) d -> p ko d", p=P)
    out_v = out.ap().rearrange("(c p) d -> p c d", p=P)          # [128, C//128, D]

    KD = D // P          # 8  k-subtiles for mm1
    KH = H // P          # 32 k-subtiles for mm2 (and h-blocks of mm1 output)
    DN = D // CHUNK      # 2 output-column blocks

    chunks = [CHUNK] * (C // CHUNK)
    if C % CHUNK:
        chunks.append(C % CHUNK)

    with tile.TileContext(nc) as tc:
        with ExitStack() as ctx:
            const = ctx.enter_context(tc.tile_pool(name="const", bufs=1))
            xpool = ctx.enter_context(tc.tile_pool(name="xp", bufs=2))
            apool = ctx.enter_context(tc.tile_pool(name="apool", bufs=1))
            tpool = ctx.enter_context(tc.tile_pool(name="tp", bufs=2))
            opool = ctx.enter_context(tc.tile_pool(name="op", bufs=4))
            ps1pool = ctx.enter_context(tc.tile_pool(name="ps1", bufs=4, space="PSUM"))
            ps2pool = ctx.enter_context(tc.tile_pool(name="ps2", bufs=3, space="PSUM"))
            warmpool = ctx.enter_context(tc.tile_pool(name="wm", bufs=1, space="PSUM"))

            # Startup-critical DMAs first. DGE descriptor generation is a
            # serialized per-path resource, so use few, large transfers and
            # spread them over all three DGE paths: SP (sync), Activation
            # (scalar) and SWDGE (gpsimd). mm1 h-block mh of chunk 0 reads
            # (in order) x8+wfc8 cols of mh, xr8, then wfcr8 cols of mh.
            x8_tiles = {}
            xr8_tiles = {}
            x8_tiles[0] = xpool.tile([P, KD, chunks[0]], f8, tag="x8", name="x8t0")
            xr8_tiles[0] = xpool.tile([P, KD, chunks[0]], f8, tag="xr8", name="xr8t0")
            wfc8_sb = const.tile([P, KD, H], f8)
            wfcr8_sb = const.tile([P, KD, H], f8)
            nc.sync.dma_start(wfc8_sb[:, :, 0:P], wfc8_v[:, :, 0:P])
            nc.scalar.dma_start(wfcr8_sb[:, :, 0:P], wfcr8_v[:, :, 0:P])
            nc.sync.dma_start(x8_tiles[0][:, 0:4, :], x8T_v[:, 0:4, 0:chunks[0]])
            nc.scalar.dma_start(xr8_tiles[0][:, 0:4, :], xr8T_v[:, 0:4, 0:chunks[0]])
            nc.sync.dma_start(x8_tiles[0][:, 4:KD, :], x8T_v[:, 4:KD, 0:chunks[0]])
            nc.scalar.dma_start(xr8_tiles[0][:, 4:KD, :], xr8T_v[:, 4:KD, 0:chunks[0]])
            # Rest of wfc8/wfcr8 in H-slices, finer early (tighter deadline),
            # coarser later. The HWDGE descriptor-generation path and the
            # DMA transfer pipe are both serialized resources, and transfers
            # run in generation order — so keep the count moderate, keep
            # descriptors >= 512B where possible, and enqueue strictly in
            # need order (wfc stream first, then wproj).
            bounds = [P, 3 * P, 7 * P, 11 * P, 15 * P, 19 * P, 23 * P,
                      27 * P, H]
            for s0, s1 in zip(bounds[:-1], bounds[1:]):
                nc.sync.dma_start(
                    wfc8_sb[:, :, s0:s1], wfc8_v[:, :, s0:s1]
                )
                nc.scalar.dma_start(
                    wfcr8_sb[:, :, s0:s1], wfcr8_v[:, :, s0:s1]
                )
            # wproj is needed once mm2 of chunk 0 starts (~41us in); its
            # generation queues behind the wfc slices, so its transfers
            # naturally follow the startup-critical stream. dn=0 halves
            # first (earlier deadline).
            wp8_sb = const.tile([P, KH, D], f8)
            wpr8_sb = const.tile([P, KH, D], f8)
            g_sb = const.tile([P, C // P], f32)
            nc.sync.dma_start(wp8_sb[:, :, 0:CHUNK], wp8_v[:, :, 0:CHUNK])
            nc.scalar.dma_start(wpr8_sb[:, :, 0:CHUNK], wpr8_v[:, :, 0:CHUNK])
            nc.sync.dma_start(wp8_sb[:, :, CHUNK:D], wp8_v[:, :, CHUNK:D])
            nc.scalar.dma_start(wpr8_sb[:, :, CHUNK:D], wpr8_v[:, :, CHUNK:D])
            nc.gpsimd.dma_start(g_sb[:], g.ap())

            # PE warmup: the HAM clock-gate needs ~3.4us of sustained matmul
            # activity to grant the 2.4 GHz rate. The PE is idle waiting for
            # the first DMAs anyway, so burn that window on dummy matmuls
            # over a zeroed scratch tile (results never read).
            warm_sb = const.tile([P, P], f16)
            nc.vector.memset(warm_sb[:], 0.0)
            warm_ps = warmpool.tile([P, P], f32)
            for _ in range(38):
                nc.tensor.matmul(warm_ps[:], warm_sb[:], warm_sb[:],
                                 start=True, stop=True)

            tok0 = 0
            for c, S in enumerate(chunks):
                x8_t = x8_tiles[c]
                xr8_t = xr8_tiles[c]
                a8_t = apool.tile([P, KH, S], f8, tag="a8")
                ar8_t = apool.tile([P, KH, S], f8, tag="ar8")
                for mh in range(KH):
                    ps1 = ps1pool.tile([P, S], f32, tag="ps1")
                    mcols = slice(mh * P, (mh + 1) * P)
                    for k in range(0, KD, 2):
                        nc.tensor.matmul(
                            ps1[:], wfc8_sb[:, k:k + 2, mcols],
                            x8_t[:, k:k + 2, :],
                            start=(k == 0), stop=False, perf_mode=DR,
                        )
                    for k in range(0, KD, 2):
                        nc.tensor.matmul(
                            ps1[:], wfc8_sb[:, k:k + 2, mcols],
                            xr8_t[:, k:k + 2, :],
                            start=False, stop=False, perf_mode=DR,
                        )
                    for k in range(0, KD, 2):
                        nc.tensor.matmul(
                            ps1[:], wfcr8_sb[:, k:k + 2, mcols],
                            x8_t[:, k:k + 2, :],
                            start=False, stop=(k == KD - 2), perf_mode=DR,
                        )
                    # t = prelu(psum/32, 0.5); af = t*t; a8 = fp8(af);
                    # ar8 = fp8(af - a8)
                    t_t = tpool.tile([P, S], f16, tag="t")
                    nc.scalar.activation(t_t[:], ps1[:], AF.Prelu,
                                         alpha=0.5, scale=1.0 / SW)
                    af_t = tpool.tile([P, S], f16, tag="af")
                    nc.vector.tensor_tensor(af_t[:], t_t[:], t_t[:], MUL)
                    nc.scalar.activation(a8_t[:, mh, :], af_t[:], AF.Copy)
                    nc.vector.tensor_tensor(
                        ar8_t[:, mh, :], af_t[:], a8_t[:, mh, :], SUB
                    )
                # Prefetch next chunk's tokens now, ahead of this chunk's
                # out-DMAs in the DGE queues.
                if c + 1 < len(chunks):
                    S1 = chunks[c + 1]
                    t1 = tok0 + S
                    x8_tiles[c + 1] = xpool.tile(
                        [P, KD, S1], f8, tag="x8", name=f"x8t{c + 1}"
                    )
                    xr8_tiles[c + 1] = xpool.tile(
                        [P, KD, S1], f8, tag="xr8", name=f"xr8t{c + 1}"
                    )
                    nc.sync.dma_start(x8_tiles[c + 1][:], x8T_v[:, :, t1:t1 + S1])
                    nc.scalar.dma_start(xr8_tiles[c + 1][:], xr8T_v[:, :, t1:t1 + S1])
                for ti in range(S // P):
                    gcol = tok0 // P + ti
                    tcols = slice(ti * P, (ti + 1) * P)
                    for dn in range(DN):
                        dcols = slice(dn * CHUNK, (dn + 1) * CHUNK)
                        ps2 = ps2pool.tile([P, CHUNK], f32, tag="ps2")
                        for k in range(0, KH, 2):
                            nc.tensor.matmul(
                                ps2[:], a8_t[:, k:k + 2, tcols],
                                wp8_sb[:, k:k + 2, dcols],
                                start=(k == 0), stop=False, perf_mode=DR,
                            )
                        for k in range(0, KH, 2):
                            nc.tensor.matmul(
                                ps2[:], ar8_t[:, k:k + 2, tcols],
                                wp8_sb[:, k:k + 2, dcols],
                                start=False, stop=False, perf_mode=DR,
                            )
                        for k in range(0, KH, 2):
                            nc.tensor.matmul(
                                ps2[:], a8_t[:, k:k + 2, tcols],
                                wpr8_sb[:, k:k + 2, dcols],
                                start=False, stop=(k == KH - 2), perf_mode=DR,
                            )
                        o_tile = opool.tile([P, CHUNK], f32, tag="ot")
                        # fused gate+descale: out = psum * (g[token]/32)
                        nc.scalar.activation(
                            o_tile[:], ps2[:], AF.Copy,
                            scale=g_sb[:, gcol:gcol + 1],
                        )
                        nc.sync.dma_start(
                            out_v[:, gcol, dcols], o_tile[:]
                        )
                tok0 += S
    nc.compile()
    _NC_CACHE[C] = nc
    return nc


def _route(xf, Wg):
    """Exact top-2 gating in fp32, mirroring the reference math."""
    logits = xf @ Wg.T                                   # [N, E]
    top2 = np.argpartition(logits, E - 2, axis=1)[:, E - 2:]   # [N, 2] unordered
    vals = np.take_along_axis(logits, top2, axis=1)
    m = vals.max(axis=1, keepdims=True)
    ex = np.exp(vals - m)
    w = ex / ex.sum(axis=1, keepdims=True)               # [N, 2] softmax over top-2
    return top2, w


def _q8(v):
    """fp8(e4m3) base + residual split of a fp32 array."""
    base = v.astype(E4NP)
    resid = (v - base.astype(np.float32)).astype(E4NP)
    return base, resid


def run_moe(x, Wg, Wfc, Wproj, trace=False):
    from concourse import bass_utils

    xf = np.ascontiguousarray(x.reshape(-1, D), dtype=np.float32)
    top2, w = _route(xf, Wg.astype(np.float32))

    toks, gates = [], []
    for e in range(E):
        sel = np.nonzero((top2 == e).any(axis=1))[0]
        ge = (w[sel] * (top2[sel] == e)).sum(axis=1).astype(np.float32)
        toks.append(sel)
        gates.append(ge)

    maxc = max(len(t) for t in toks)
    C = max(P, ((maxc + P - 1) // P) * P)

    nc = _build_nc(C)

    x8_full, xr8_full = _q8(xf)
    in_maps = []
    for e in range(E):
        te = toks[e]
        x8T_e = np.zeros((D, C), E4NP)
        x8T_e[:, :len(te)] = x8_full[te].T
        xr8T_e = np.zeros((D, C), E4NP)
        xr8T_e[:, :len(te)] = xr8_full[te].T
        wfc8, wfcr8 = _q8(np.ascontiguousarray(Wfc[e].T, np.float32) * SW)
        wp8, wpr8 = _q8(np.ascontiguousarray(Wproj[e].T, np.float32) * SW)
        g_e = np.zeros((C,), np.float32)
        g_e[:len(te)] = gates[e] / SW
        g_mat = np.ascontiguousarray(g_e.reshape(C // P, P).T)
        in_maps.append({
            "x8T": x8T_e,
            "xr8T": xr8T_e,
            "wfc8T": wfc8,
            "wfcr8T": wfcr8,
            "wp8T": wp8,
            "wpr8T": wpr8,
            "g": g_mat,
        })

    # NTFF tracing is unavailable under this axon environment (no
    # antenv.axon_hooks); always run untraced.
    res = bass_utils.run_bass_kernel_spmd(
        nc, in_maps, core_ids=list(range(E)), trace=False
    )

    out = np.zeros((N, D), np.float32)
    for e in range(E):
        te = toks[e]
        out[te] += res.results[e]["outp"][:len(te)]
    return out.reshape(B, T, D), res


def kernel(x, Wg, Wfc, Wproj):
    out, _ = run_moe(np.asarray(x), np.asarray(Wg), np.asarray(Wfc), np.asarray(Wproj))
    return out


# revision 14
# speedup vs baseline: 1.0723x; 1.0149x over previous
"""MoE MLP (top-2 of 8 experts) Trainium2 kernel.

Strategy: expert-parallel across the 8 NeuronCores (host computes the tiny
top-2 gating exactly in fp32 and gathers each expert's tokens into a
capacity-padded buffer; core e runs expert e's two big matmuls).

The matmuls run as fp8(e4m3) DoubleRow matmuls with a 3-term residual
decomposition. Each operand is split into an fp8 base plus an fp8 residual
(quantization error of the base), and the product keeps the three large
cross-terms:

    x @ W  ~=  x8@W8 + x8@Wr8 + xr8@W8        (drop xr8@Wr8, ~2^-8 relative)

DoubleRow pairs two 128-row contraction subtiles per PE instruction at half
the per-column cost, so the 3-term product costs 0.75x of one bf16 matmul
while keeping ~2^-8 effective precision (measured end-to-end rel err ~2e-3).

Weights are pre-scaled by 32 before fp8 quantization so their ~0.02-scale
entries sit in e4m3's normal range; the 1/32 descale folds into the mm1
activation scale and the per-token gate (g/32) on mm2 eviction.

Device layout keeps the contraction dim on SBUF partitions throughout:
  mm1: psum[h_blk(128), tok] += DR-pairs of {wfc8,wfcr8}[d,h_blk] x {x8,xr8}[d,tok]
  act: t = prelu(psum/32, 0.5) (fp16); af = t*t (fp16);
       a8 = fp8(af); ar8 = fp8(af - a8)
  mm2: psum[tok(128), d(512)] += DR-pairs of {a8,ar8}[h,tok] x {wp8,wpr8}[h,d]
  evict: out = psum * (g[token]/32)

Host scatters per-expert rows back (each token appears in exactly 2 expert
lists) and sums - identical math to the reference's dense masked combine.
"""

import numpy as np
import ml_dtypes
from contextlib import ExitStack

B, T, D, H, E = 4, 2048, 1024, 4096, 8
N = B * T
P = 128
CHUNK = 512
SW = 32.0  # weight pre-scale before fp8 quantization

E4NP = ml_dtypes.float8_e4m3  # == concourse mybir.dt.float8e4


_NC_CACHE = {}


def _build_nc(C):
    """Build + compile the per-core Bass program for capacity C tokens.

    C must be a multiple of 128. Tokens stream in chunks of 512 plus one
    optional tail chunk of C % 512. The program depends only on C, so it is
    cached: reusing the same nc object also lets bass2jax's jit cache skip
    the NEFF compile on repeat kernel() calls.
    """
    if C in _NC_CACHE:
        return _NC_CACHE[C]
    import concourse.bacc as bacc
    import concourse.tile as tile
    import concourse.mybir as mybir

    assert C % P == 0
    f8 = mybir.dt.float8e4
    f16 = mybir.dt.float16
    f32 = mybir.dt.float32
    AF = mybir.ActivationFunctionType
    DR = mybir.MatmulPerfMode.DoubleRow
    MUL = mybir.AluOpType.mult
    SUB = mybir.AluOpType.subtract

    nc = bacc.Bacc(None, target_bir_lowering=False, debug=False)
    x8T = nc.dram_tensor("x8T", [D, C], f8, kind="ExternalInput")
    xr8T = nc.dram_tensor("xr8T", [D, C], f8, kind="ExternalInput")
    wfc8T = nc.dram_tensor("wfc8T", [D, H], f8, kind="ExternalInput")
    wfcr8T = nc.dram_tensor("wfcr8T", [D, H], f8, kind="ExternalInput")
    wp8T = nc.dram_tensor("wp8T", [H, D], f8, kind="ExternalInput")
    wpr8T = nc.dram_tensor("wpr8T", [H, D], f8, kind="ExternalInput")
    g = nc.dram_tensor("g", [P, C // P], f32, kind="ExternalInput")
    out = nc.dram_tensor("outp", [C, D], f32, kind="ExternalOutput")

    x8T_v = x8T.ap().rearrange("(ko p) c -> p ko c", p=P)        # [128, 8, C]
    xr8T_v = xr8T.ap().rearrange("(ko p) c -> p ko c", p=P)
    wfc8_v = wfc8T.ap().rearrange("(ko p) h -> p ko h", p=P)     # [128, 8, H]
    wfcr8_v = wfcr8T.ap().rearrange("(ko p) h -> p ko h", p=P)
    wp8_v = wp8T.ap().rearrange("(ko p) d -> p ko d", p=P)       # [128, 32, D]
    wpr8_v = wpr8T.ap().rearrange("(ko p) d -> p ko d", p=P)
    out_v = out.ap().rearrange("(c p) d -> p c d", p=P)          # [128, C//128, D]

    KD = D // P          # 8  k-subtiles for mm1
    KH = H // P          # 32 k-subtiles for mm2 (and h-blocks of mm1 output)
    DN = D // CHUNK      # 2 output-column blocks

    chunks = [CHUNK] * (C // CHUNK)
    if C % CHUNK:
        chunks.append(C % CHUNK)
    # A short tail chunk (<256 tokens) stalls the PE on activation-chain
    # latency (mm1 h-blocks outpace the prelu/square/quantize pipeline);
    # rebalance the last two chunks to keep every chunk >= 256 tokens.
    if len(chunks) >= 2 and chunks[-1] < 2 * P:
        pair = chunks[-2] + chunks[-1]
        a = (pair // 2) // P * P
        chunks[-2:] = [a, pair - a]

    with tile.TileContext(nc) as tc:
        with ExitStack() as ctx:
            const = ctx.enter_context(tc.tile_pool(name="const", bufs=1))
            xpool = ctx.enter_context(tc.tile_pool(name="xp", bufs=2))
            apool = ctx.enter_context(tc.tile_pool(name="apool", bufs=1))
            tpool = ctx.enter_context(tc.tile_pool(name="tp", bufs=2))
            opool = ctx.enter_context(tc.tile_pool(name="op", bufs=4))
            ps1pool = ctx.enter_context(tc.tile_pool(name="ps1", bufs=4, space="PSUM"))
            ps2pool = ctx.enter_context(tc.tile_pool(name="ps2", bufs=3, space="PSUM"))
            warmpool = ctx.enter_context(tc.tile_pool(name="wm", bufs=1, space="PSUM"))

            # Startup-critical DMAs first. DGE descriptor generation is a
            # serialized per-path resource, so use few, large transfers and
            # spread them over all three DGE paths: SP (sync), Activation
            # (scalar) and SWDGE (gpsimd). mm1 h-block mh of chunk 0 reads
            # (in order) x8+wfc8 cols of mh, xr8, then wfcr8 cols of mh.
            x8_tiles = {}
            xr8_tiles = {}
            x8_tiles[0] = xpool.tile([P, KD, chunks[0]], f8, tag="x8", name="x8t0")
            xr8_tiles[0] = xpool.tile([P, KD, chunks[0]], f8, tag="xr8", name="xr8t0")
            wfc8_sb = const.tile([P, KD, H], f8)
            wfcr8_sb = const.tile([P, KD, H], f8)
            # The HWDGE descriptor-generation path (shared by SP+Activation)
            # and the DMA transfer pipe are both serialized resources, and
            # transfers run in generation order. Rules: (1) no DMAs on the
            # Activation engine — its in-order SEQ would stall the act chain
            # behind DMA generation; (2) the handful of immediately-needed
            # transfers go via the Pool-engine SWDGE (parallel generation);
            # (3) everything else goes on SP strictly in need order.
            nc.gpsimd.dma_start(wfc8_sb[:, :, 0:P], wfc8_v[:, :, 0:P])
            nc.gpsimd.dma_start(wfcr8_sb[:, :, 0:P], wfcr8_v[:, :, 0:P])
            nc.gpsimd.dma_start(x8_tiles[0][:, 0:4, :], x8T_v[:, 0:4, 0:chunks[0]])
            nc.gpsimd.dma_start(xr8_tiles[0][:, 0:4, :], xr8T_v[:, 0:4, 0:chunks[0]])
            nc.gpsimd.dma_start(x8_tiles[0][:, 4:KD, :], x8T_v[:, 4:KD, 0:chunks[0]])
            nc.gpsimd.dma_start(xr8_tiles[0][:, 4:KD, :], xr8T_v[:, 4:KD, 0:chunks[0]])
            # Rest of wfc8/wfcr8 in H-slices, finer early (tighter deadline),
            # coarser later: mm1 h-block mh only depends on the slices
            # covering its 128 cols (subregion deps).
            bounds = [P, 3 * P, 7 * P, 11 * P, 15 * P, 19 * P, 23 * P,
                      27 * P, H]
            for s0, s1 in zip(bounds[:-1], bounds[1:]):
                nc.sync.dma_start(
                    wfc8_sb[:, :, s0:s1], wfc8_v[:, :, s0:s1]
                )
                nc.sync.dma_start(
                    wfcr8_sb[:, :, s0:s1], wfcr8_v[:, :, s0:s1]
                )
            # wproj is needed once mm2 of chunk 0 starts (~41us in); its
            # generation queues behind the wfc slices, so its transfers
            # naturally follow the startup-critical stream. dn=0 halves
            # first (earlier deadline); the dn=1 halves are emitted in the
            # chunk-0 body after the chunk-1 token prefetch.
            wp8_sb = const.tile([P, KH, D], f8)
            wpr8_sb = const.tile([P, KH, D], f8)
            g_sb = const.tile([P, C // P], f32)
            nc.sync.dma_start(wp8_sb[:, :, 0:CHUNK], wp8_v[:, :, 0:CHUNK])
            nc.sync.dma_start(wpr8_sb[:, :, 0:CHUNK], wpr8_v[:, :, 0:CHUNK])
            nc.gpsimd.dma_start(g_sb[:], g.ap())

            # PE warmup: the HAM clock-gate needs ~3.4us of sustained matmul
            # activity to grant the 2.4 GHz rate. The PE is idle waiting for
            # the first DMAs anyway, so burn that window on dummy matmuls
            # over a zeroed scratch tile (results never read).
            warm_sb = const.tile([P, P], f16)
            nc.vector.memset(warm_sb[:], 0.0)
            warm_ps = warmpool.tile([P, P], f32)
            for _ in range(34):
                nc.tensor.matmul(warm_ps[:], warm_sb[:], warm_sb[:],
                                 start=True, stop=True)

            tok0 = 0
            for c, S in enumerate(chunks):
                x8_t = x8_tiles[c]
                xr8_t = xr8_tiles[c]
                a8_t = apool.tile([P, KH, S], f8, tag="a8")
                ar8_t = apool.tile([P, KH, S], f8, tag="ar8")
                for mh in range(KH):
                    ps1 = ps1pool.tile([P, S], f32, tag="ps1")
                    mcols = slice(mh * P, (mh + 1) * P)
                    for k in range(0, KD, 2):
                        nc.tensor.matmul(
                            ps1[:], wfc8_sb[:, k:k + 2, mcols],
                            x8_t[:, k:k + 2, :],
                            start=(k == 0), stop=False, perf_mode=DR,
                        )
                    for k in range(0, KD, 2):
                        nc.tensor.matmul(
                            ps1[:], wfc8_sb[:, k:k + 2, mcols],
                            xr8_t[:, k:k + 2, :],
                            start=False, stop=False, perf_mode=DR,
                        )
                    for k in range(0, KD, 2):
                        nc.tensor.matmul(
                            ps1[:], wfcr8_sb[:, k:k + 2, mcols],
                            x8_t[:, k:k + 2, :],
                            start=False, stop=(k == KD - 2), perf_mode=DR,
                        )
                    # t = prelu(psum/32, 0.5); af = t*t; a8 = fp8(af);
                    # ar8 = fp8(af - a8)
                    t_t = tpool.tile([P, S], f16, tag="t")
                    nc.scalar.activation(t_t[:], ps1[:], AF.Prelu,
                                         alpha=0.5, scale=1.0 / SW)
                    af_t = tpool.tile([P, S], f16, tag="af")
                    nc.vector.tensor_tensor(af_t[:], t_t[:], t_t[:], MUL)
                    nc.scalar.activation(a8_t[:, mh, :], af_t[:], AF.Copy)
                    nc.vector.tensor_tensor(
                        ar8_t[:, mh, :], af_t[:], a8_t[:, mh, :], SUB
                    )
                # Prefetch next chunk's tokens now, ahead of this chunk's
                # out-DMAs in the DGE queues.
                if c + 1 < len(chunks):
                    S1 = chunks[c + 1]
                    t1 = tok0 + S
                    x8_tiles[c + 1] = xpool.tile(
                        [P, KD, S1], f8, tag="x8", name=f"x8t{c + 1}"
                    )
                    xr8_tiles[c + 1] = xpool.tile(
                        [P, KD, S1], f8, tag="xr8", name=f"xr8t{c + 1}"
                    )
                    nc.sync.dma_start(x8_tiles[c + 1][:], x8T_v[:, :, t1:t1 + S1])
                    nc.sync.dma_start(xr8_tiles[c + 1][:], xr8T_v[:, :, t1:t1 + S1])
                if c == 0:
                    nc.sync.dma_start(wp8_sb[:, :, CHUNK:D], wp8_v[:, :, CHUNK:D])
                    nc.sync.dma_start(wpr8_sb[:, :, CHUNK:D], wpr8_v[:, :, CHUNK:D])
                for ti in range(S // P):
                    gcol = tok0 // P + ti
                    tcols = slice(ti * P, (ti + 1) * P)
                    for dn in range(DN):
                        dcols = slice(dn * CHUNK, (dn + 1) * CHUNK)
                        ps2 = ps2pool.tile([P, CHUNK], f32, tag="ps2")
                        for k in range(0, KH, 2):
                            nc.tensor.matmul(
                                ps2[:], a8_t[:, k:k + 2, tcols],
                                wp8_sb[:, k:k + 2, dcols],
                                start=(k == 0), stop=False, perf_mode=DR,
                            )
                        for k in range(0, KH, 2):
                            nc.tensor.matmul(
                                ps2[:], ar8_t[:, k:k + 2, tcols],
                                wp8_sb[:, k:k + 2, dcols],
                                start=False, stop=False, perf_mode=DR,
                            )
                        for k in range(0, KH, 2):
                            nc.tensor.matmul(
                                ps2[:], a8_t[:, k:k + 2, tcols],
                                wpr8_sb[:, k:k + 2, dcols],
                                start=False, stop=(k == KH - 2), perf_mode=DR,
                            )
                        o_tile = opool.tile([P, CHUNK], f32, tag="ot")
                        # fused gate+descale: out = psum * (g[token]/32)
                        nc.scalar.activation(
                            o_tile[:], ps2[:], AF.Copy,
                            scale=g_sb[:, gcol:gcol + 1],
                        )
                        nc.sync.dma_start(
                            out_v[:, gcol, dcols], o_tile[:]
                        )
                tok0 += S
    nc.compile()
    _NC_CACHE[C] = nc
    return nc


def _route(xf, Wg):
    """Exact top-2 gating in fp32, mirroring the reference math."""
    logits = xf @ Wg.T                                   # [N, E]
    top2 = np.argpartition(logits, E - 2, axis=1)[:, E - 2:]   # [N, 2] unordered
    vals = np.take_along_axis(logits, top2, axis=1)
    m = vals.max(axis=1, keepdims=True)
    ex = np.exp(vals - m)
    w = ex / ex.sum(axis=1, keepdims=True)               # [N, 2] softmax over top-2
    return top2, w


def _q8(v):
    """fp8(e4m3) base + residual split of a fp32 array."""
    base = v.astype(E4NP)
    resid = (v - base.astype(np.float32)).astype(E4NP)
    return base, resid


def run_moe(x, Wg, Wfc, Wproj, trace=False):
    from concourse import bass_utils

    xf = np.ascontiguousarray(x.reshape(-1, D), dtype=np.float32)
    top2, w = _route(xf, Wg.astype(np.float32))

    toks, gates = [], []
    for e in range(E):
        sel = np.nonzero((top2 == e).any(axis=1))[0]
        ge = (w[sel] * (top2[sel] == e)).sum(axis=1).astype(np.float32)
        toks.append(sel)
        gates.append(ge)

    maxc = max(len(t) for t in toks)
    C = max(P, ((maxc + P - 1) // P) * P)

    nc = _build_nc(C)

    x8_full, xr8_full = _q8(xf)
    in_maps = []
    for e in range(E):
        te = toks[e]
        x8T_e = np.zeros((D, C), E4NP)
        x8T_e[:, :len(te)] = x8_full[te].T
        xr8T_e = np.zeros((D, C), E4NP)
        xr8T_e[:, :len(te)] = xr8_full[te].T
        wfc8, wfcr8 = _q8(np.ascontiguousarray(Wfc[e].T, np.float32) * SW)
        wp8, wpr8 = _q8(np.ascontiguousarray(Wproj[e].T, np.float32) * SW)
        g_e = np.zeros((C,), np.float32)
        g_e[:len(te)] = gates[e] / SW
        g_mat = np.ascontiguousarray(g_e.reshape(C // P, P).T)
        in_maps.append({
            "x8T": x8T_e,
            "xr8T": xr8T_e,
            "wfc8T": wfc8,
            "wfcr8T": wfcr8,
            "wp8T": wp8,
            "wpr8T": wpr8,
            "g": g_mat,
        })

    # NTFF tracing is unavailable under this axon environment (no
    # antenv.axon_hooks); always run untraced.
    res = bass_utils.run_bass_kernel_spmd(
        nc, in_maps, core_ids=list(range(E)), trace=False
    )

    out = np.zeros((N, D), np.float32)
    for e in range(E):
        te = toks[e]
        out[te] += res.results[e]["outp"][:len(te)]
    return out.reshape(B, T, D), res


def kernel(x, Wg, Wfc, Wproj):
    out, _ = run_moe(np.asarray(x), np.asarray(Wg), np.asarray(Wfc), np.asarray(Wproj))
    return out


# revision 18
# speedup vs baseline: 1.1187x; 1.0433x over previous
"""MoE MLP (top-2 of 8 experts) Trainium2 kernel.

Strategy: expert-parallel across the 8 NeuronCores (host computes the tiny
top-2 gating exactly in fp32 and gathers each expert's tokens into a
capacity-padded buffer; core e runs expert e's two big matmuls).

The matmuls run as fp8(e4m3) DoubleRow matmuls with a 3-term residual
decomposition. Each operand is split into an fp8 base plus an fp8 residual
(quantization error of the base), and the product keeps the three large
cross-terms:

    x @ W  ~=  x8@W8 + x8@Wr8 + xr8@W8        (drop xr8@Wr8, ~2^-8 relative)

DoubleRow pairs two 128-row contraction subtiles per PE instruction at half
the per-column cost, so the 3-term product costs 0.75x of one bf16 matmul
while keeping ~2^-8 effective precision (measured end-to-end rel err ~2e-3).

Weights are pre-scaled by 32 before fp8 quantization so their ~0.02-scale
entries sit in e4m3's normal range; the 1/32 descale folds into the mm1
activation scale and the per-token gate (g/32) on mm2 eviction.

Device layout keeps the contraction dim on SBUF partitions throughout:
  mm1: psum[h_blk(128), tok] += DR-pairs of {wfc8,wfcr8}[d,h_blk] x {x8,xr8}[d,tok]
  act: t = prelu(psum/32, 0.5) (fp16); af = t*t (fp16);
       a8 = fp8(af); ar8 = fp8(af - a8)
  mm2: psum[tok(128), d(512)] += DR-pairs of {a8,ar8}[h,tok] x {wp8,wpr8}[h,d]
  evict: out = psum * (g[token]/32)

Host scatters per-expert rows back (each token appears in exactly 2 expert
lists) and sums - identical math to the reference's dense masked combine.
"""

import numpy as np
import ml_dtypes
from contextlib import ExitStack

B, T, D, H, E = 4, 2048, 1024, 4096, 8
N = B * T
P = 128
CHUNK = 512
SW = 32.0  # weight pre-scale before fp8 quantization

E4NP = ml_dtypes.float8_e4m3  # == concourse mybir.dt.float8e4


_NC_CACHE = {}


def _build_nc(C):
    """Build + compile the per-core Bass program for capacity C tokens.

    C must be a multiple of 128. Tokens stream in chunks of 512 plus one
    optional tail chunk of C % 512. The program depends only on C, so it is
    cached: reusing the same nc object also lets bass2jax's jit cache skip
    the NEFF compile on repeat kernel() calls.
    """
    if C in _NC_CACHE:
        return _NC_CACHE[C]
    import concourse.bacc as bacc
    import concourse.tile as tile
    import concourse.mybir as mybir

    assert C % P == 0
    f8 = mybir.dt.float8e4
    f16 = mybir.dt.float16
    f32 = mybir.dt.float32
    AF = mybir.ActivationFunctionType
    DR = mybir.MatmulPerfMode.DoubleRow
    MUL = mybir.AluOpType.mult
    SUB = mybir.AluOpType.subtract

    nc = bacc.Bacc(None, target_bir_lowering=False, debug=False)
    x8T = nc.dram_tensor("x8T", [D, C], f8, kind="ExternalInput")
    xr8T = nc.dram_tensor("xr8T", [D, C], f8, kind="ExternalInput")
    wfc8T = nc.dram_tensor("wfc8T", [D, H], f8, kind="ExternalInput")
    wfcr8T = nc.dram_tensor("wfcr8T", [D, H], f8, kind="ExternalInput")
    wp8T = nc.dram_tensor("wp8T", [H, D], f8, kind="ExternalInput")
    wpr8T = nc.dram_tensor("wpr8T", [H, D], f8, kind="ExternalInput")
    g = nc.dram_tensor("g", [P, C // P], f32, kind="ExternalInput")
    out = nc.dram_tensor("outp", [C, D], f32, kind="ExternalOutput")

    x8T_v = x8T.ap().rearrange("(ko p) c -> p ko c", p=P)        # [128, 8, C]
    xr8T_v = xr8T.ap().rearrange("(ko p) c -> p ko c", p=P)
    wfc8_v = wfc8T.ap().rearrange("(ko p) h -> p ko h", p=P)     # [128, 8, H]
    wfcr8_v = wfcr8T.ap().rearrange("(ko p) h -> p ko h", p=P)
    wp8_v = wp8T.ap().rearrange("(ko p) d -> p ko d", p=P)       # [128, 32, D]
    wpr8_v = wpr8T.ap().rearrange("(ko p) d -> p ko d", p=P)
    out_v = out.ap().rearrange("(c p) d -> p c d", p=P)          # [128, C//128, D]

    KD = D // P          # 8  k-subtiles for mm1
    KH = H // P          # 32 k-subtiles for mm2 (and h-blocks of mm1 output)
    DN = D // CHUNK      # 2 output-column blocks

    chunks = [CHUNK] * (C // CHUNK)
    if C % CHUNK:
        chunks.append(C % CHUNK)
    # A short tail chunk (<256 tokens) stalls the PE on activation-chain
    # latency (mm1 h-blocks outpace the prelu/square/quantize pipeline);
    # rebalance the last two chunks to keep every chunk >= 256 tokens.
    if len(chunks) >= 2 and chunks[-1] < 2 * P:
        pair = chunks[-2] + chunks[-1]
        a = (pair // 2) // P * P
        chunks[-2:] = [a, pair - a]

    with tile.TileContext(nc) as tc:
        with ExitStack() as ctx:
            const = ctx.enter_context(tc.tile_pool(name="const", bufs=1))
            xpool = ctx.enter_context(tc.tile_pool(name="xp", bufs=2))
            apool = ctx.enter_context(tc.tile_pool(name="apool", bufs=1))
            tpool = ctx.enter_context(tc.tile_pool(name="tp", bufs=2))
            opool = ctx.enter_context(tc.tile_pool(name="op", bufs=4))
            ps1pool = ctx.enter_context(tc.tile_pool(name="ps1", bufs=4, space="PSUM"))
            ps2pool = ctx.enter_context(tc.tile_pool(name="ps2", bufs=3, space="PSUM"))
            warmpool = ctx.enter_context(tc.tile_pool(name="wm", bufs=1, space="PSUM"))

            # Startup-critical DMAs first. DGE descriptor generation is a
            # serialized per-path resource, so use few, large transfers and
            # spread them over all three DGE paths: SP (sync), Activation
            # (scalar) and SWDGE (gpsimd). mm1 h-block mh of chunk 0 reads
            # (in order) x8+wfc8 cols of mh, xr8, then wfcr8 cols of mh.
            x8_tiles = {}
            xr8_tiles = {}
            x8_tiles[0] = xpool.tile([P, KD, chunks[0]], f8, tag="x8", name="x8t0")
            xr8_tiles[0] = xpool.tile([P, KD, chunks[0]], f8, tag="xr8", name="xr8t0")
            wfc8_sb = const.tile([P, KD, H], f8)
            wfcr8_sb = const.tile([P, KD, H], f8)
            # The HWDGE descriptor-generation path (shared by SP+Activation)
            # and the DMA transfer pipe are both serialized resources, and
            # transfers run in generation order. Rules: (1) no DMAs on the
            # Activation engine — its in-order SEQ would stall the act chain
            # behind DMA generation; (2) the handful of immediately-needed
            # transfers go via the Pool-engine SWDGE (parallel generation);
            # (3) everything else goes on SP strictly in need order.
            nc.sync.dma_start(wfc8_sb[:, :, 0:P], wfc8_v[:, :, 0:P])
            nc.sync.dma_start(x8_tiles[0][:, 0:4, :], x8T_v[:, 0:4, 0:chunks[0]])
            nc.sync.dma_start(wfcr8_sb[:, :, 0:P], wfcr8_v[:, :, 0:P])
            nc.sync.dma_start(xr8_tiles[0][:, 0:4, :], xr8T_v[:, 0:4, 0:chunks[0]])
            nc.sync.dma_start(x8_tiles[0][:, 4:KD, :], x8T_v[:, 4:KD, 0:chunks[0]])
            nc.sync.dma_start(xr8_tiles[0][:, 4:KD, :], xr8T_v[:, 4:KD, 0:chunks[0]])
            # Rest of wfc8/wfcr8 in H-slices, finer early (tighter deadline),
            # coarser later: mm1 h-block mh only depends on the slices
            # covering its 128 cols (subregion deps).
            bounds = [P, 3 * P, 7 * P, 11 * P, 15 * P, 19 * P, 23 * P,
                      27 * P, H]
            for s0, s1 in zip(bounds[:-1], bounds[1:]):
                nc.sync.dma_start(
                    wfc8_sb[:, :, s0:s1], wfc8_v[:, :, s0:s1]
                )
                nc.sync.dma_start(
                    wfcr8_sb[:, :, s0:s1], wfcr8_v[:, :, s0:s1]
                )
            # Chunk-1 tokens next (needed when chunk-1 mm1 starts ~46us in),
            # then wproj for mm2 of chunk 0: dn=0 halves first (earlier
            # deadline); the dn=1 halves are emitted in the chunk-0 body.
            if len(chunks) > 1:
                S1 = chunks[1]
                x8_tiles[1] = xpool.tile([P, KD, S1], f8, tag="x8", name="x8t1")
                xr8_tiles[1] = xpool.tile([P, KD, S1], f8, tag="xr8", name="xr8t1")
                nc.sync.dma_start(x8_tiles[1][:], x8T_v[:, :, chunks[0]:chunks[0] + S1])
                nc.sync.dma_start(xr8_tiles[1][:], xr8T_v[:, :, chunks[0]:chunks[0] + S1])
            wp8_sb = const.tile([P, KH, D], f8)
            wpr8_sb = const.tile([P, KH, D], f8)
            g_sb = const.tile([P, C // P], f32)
            nc.sync.dma_start(wp8_sb[:, :, 0:CHUNK], wp8_v[:, :, 0:CHUNK])
            nc.sync.dma_start(wpr8_sb[:, :, 0:CHUNK], wpr8_v[:, :, 0:CHUNK])
            nc.gpsimd.dma_start(g_sb[:], g.ap())

            # PE warmup: the HAM clock-gate needs ~3.4us of sustained matmul
            # activity to grant the 2.4 GHz rate. The PE is idle waiting for
            # the first DMAs anyway, so burn that window on dummy matmuls
            # over a zeroed scratch tile (results never read).
            warm_sb = const.tile([P, P], f16)
            nc.vector.memset(warm_sb[:], 0.0)
            warm_ps = warmpool.tile([P, P], f32)
            for _ in range(38):
                nc.tensor.matmul(warm_ps[:], warm_sb[:], warm_sb[:],
                                 start=True, stop=True)

            tok0 = 0
            for c, S in enumerate(chunks):
                x8_t = x8_tiles[c]
                xr8_t = xr8_tiles[c]
                a8_t = apool.tile([P, KH, S], f8, tag="a8")
                ar8_t = apool.tile([P, KH, S], f8, tag="ar8")
                for mh in range(KH):
                    ps1 = ps1pool.tile([P, S], f32, tag="ps1")
                    mcols = slice(mh * P, (mh + 1) * P)
                    for k in range(0, KD, 2):
                        nc.tensor.matmul(
                            ps1[:], wfc8_sb[:, k:k + 2, mcols],
                            x8_t[:, k:k + 2, :],
                            start=(k == 0), stop=False, perf_mode=DR,
                        )
                    for k in range(0, KD, 2):
                        nc.tensor.matmul(
                            ps1[:], wfc8_sb[:, k:k + 2, mcols],
                            xr8_t[:, k:k + 2, :],
                            start=False, stop=False, perf_mode=DR,
                        )
                    for k in range(0, KD, 2):
                        nc.tensor.matmul(
                            ps1[:], wfcr8_sb[:, k:k + 2, mcols],
                            x8_t[:, k:k + 2, :],
                            start=False, stop=(k == KD - 2), perf_mode=DR,
                        )
                    # t = prelu(psum/32, 0.5); af = t*t; a8 = fp8(af);
                    # ar8 = fp8(af - a8)
                    t_t = tpool.tile([P, S], f16, tag="t")
                    nc.scalar.activation(t_t[:], ps1[:], AF.Prelu,
                                         alpha=0.5, scale=1.0 / SW)
                    af_t = tpool.tile([P, S], f16, tag="af")
                    nc.vector.tensor_tensor(af_t[:], t_t[:], t_t[:], MUL)
                    nc.scalar.activation(a8_t[:, mh, :], af_t[:], AF.Copy)
                    nc.vector.tensor_tensor(
                        ar8_t[:, mh, :], af_t[:], a8_t[:, mh, :], SUB
                    )
                # Prefetch next chunk's tokens now, ahead of this chunk's
                # out-DMAs in the DGE queues.
                if c + 1 < len(chunks) and c + 1 not in x8_tiles:
                    S1 = chunks[c + 1]
                    t1 = tok0 + S
                    x8_tiles[c + 1] = xpool.tile(
                        [P, KD, S1], f8, tag="x8", name=f"x8t{c + 1}"
                    )
                    xr8_tiles[c + 1] = xpool.tile(
                        [P, KD, S1], f8, tag="xr8", name=f"xr8t{c + 1}"
                    )
                    nc.sync.dma_start(x8_tiles[c + 1][:], x8T_v[:, :, t1:t1 + S1])
                    nc.sync.dma_start(xr8_tiles[c + 1][:], xr8T_v[:, :, t1:t1 + S1])
                if c == 0:
                    nc.sync.dma_start(wp8_sb[:, :, CHUNK:D], wp8_v[:, :, CHUNK:D])
                    nc.sync.dma_start(wpr8_sb[:, :, CHUNK:D], wpr8_v[:, :, CHUNK:D])
                for ti in range(S // P):
                    gcol = tok0 // P + ti
                    tcols = slice(ti * P, (ti + 1) * P)
                    for dn in range(DN):
                        dcols = slice(dn * CHUNK, (dn + 1) * CHUNK)
                        ps2 = ps2pool.tile([P, CHUNK], f32, tag="ps2")
                        for k in range(0, KH, 2):
                            nc.tensor.matmul(
                                ps2[:], a8_t[:, k:k + 2, tcols],
                                wp8_sb[:, k:k + 2, dcols],
                                start=(k == 0), stop=False, perf_mode=DR,
                            )
                        for k in range(0, KH, 2):
                            nc.tensor.matmul(
                                ps2[:], ar8_t[:, k:k + 2, tcols],
                                wp8_sb[:, k:k + 2, dcols],
                                start=False, stop=False, perf_mode=DR,
                            )
                        for k in range(0, KH, 2):
                            nc.tensor.matmul(
                                ps2[:], a8_t[:, k:k + 2, tcols],
                                wpr8_sb[:, k:k + 2, dcols],
                                start=False, stop=(k == KH - 2), perf_mode=DR,
                            )
                        o_tile = opool.tile([P, CHUNK], f32, tag="ot")
                        # fused gate+descale: out = psum * (g[token]/32)
                        nc.scalar.activation(
                            o_tile[:], ps2[:], AF.Copy,
                            scale=g_sb[:, gcol:gcol + 1],
                        )
                        nc.sync.dma_start(
                            out_v[:, gcol, dcols], o_tile[:]
                        )
                tok0 += S
    nc.compile()
    _NC_CACHE[C] = nc
    return nc


def _route(xf, Wg):
    """Exact top-2 gating in fp32, mirroring the reference math."""
    logits = xf @ Wg.T                                   # [N, E]
    top2 = np.argpartition(logits, E - 2, axis=1)[:, E - 2:]   # [N, 2] unordered
    vals = np.take_along_axis(logits, top2, axis=1)
    m = vals.max(axis=1, keepdims=True)
    ex = np.exp(vals - m)
    w = ex / ex.sum(axis=1, keepdims=True)               # [N, 2] softmax over top-2
    return top2, w


def _q8(v):
    """fp8(e4m3) base + residual split of a fp32 array."""
    base = v.astype(E4NP)
    resid = (v - base.astype(np.float32)).astype(E4NP)
    return base, resid


def run_moe(x, Wg, Wfc, Wproj, trace=False):
    from concourse import bass_utils

    xf = np.ascontiguousarray(x.reshape(-1, D), dtype=np.float32)
    top2, w = _route(xf, Wg.astype(np.float32))

    toks, gates = [], []
    for e in range(E):
        sel = np.nonzero((top2 == e).any(axis=1))[0]
        ge = (w[sel] * (top2[sel] == e)).sum(axis=1).astype(np.float32)
        toks.append(sel)
        gates.append(ge)

    maxc = max(len(t) for t in toks)
    C = max(P, ((maxc + P - 1) // P) * P)

    nc = _build_nc(C)

    x8_full, xr8_full = _q8(xf)
    in_maps = []
    for e in range(E):
        te = toks[e]
        x8T_e = np.zeros((D, C), E4NP)
        x8T_e[:, :len(te)] = x8_full[te].T
        xr8T_e = np.zeros((D, C), E4NP)
        xr8T_e[:, :len(te)] = xr8_full[te].T
        wfc8, wfcr8 = _q8(np.ascontiguousarray(Wfc[e].T, np.float32) * SW)
        wp8, wpr8 = _q8(np.ascontiguousarray(Wproj[e].T, np.float32) * SW)
        g_e = np.zeros((C,), np.float32)
        g_e[:len(te)] = gates[e] / SW
        g_mat = np.ascontiguousarray(g_e.reshape(C // P, P).T)
        in_maps.append({
            "x8T": x8T_e,
            "xr8T": xr8T_e,
            "wfc8T": wfc8,
            "wfcr8T": wfcr8,
            "wp8T": wp8,
            "wpr8T": wpr8,
            "g": g_mat,
        })

    # NTFF tracing is unavailable under this axon environment (no
    # antenv.axon_hooks); always run untraced.
    res = bass_utils.run_bass_kernel_spmd(
        nc, in_maps, core_ids=list(range(E)), trace=False
    )

    out = np.zeros((N, D), np.float32)
    for e in range(E):
        te = toks[e]
        out[te] += res.results[e]["outp"][:len(te)]
    return out.reshape(B, T, D), res


def kernel(x, Wg, Wfc, Wproj):
    out, _ = run_moe(np.asarray(x), np.asarray(Wg), np.asarray(Wfc), np.asarray(Wproj))
    return out


# revision 20
# speedup vs baseline: 1.1188x; 1.0000x over previous
"""MoE MLP (top-2 of 8 experts) Trainium2 kernel.

Strategy: expert-parallel across the 8 NeuronCores (host computes the tiny
top-2 gating exactly in fp32 and gathers each expert's tokens into a
capacity-padded buffer; core e runs expert e's two big matmuls).

The matmuls run as fp8(e4m3) DoubleRow matmuls with a 3-term residual
decomposition. Each operand is split into an fp8 base plus an fp8 residual
(quantization error of the base), and the product keeps the three large
cross-terms:

    x @ W  ~=  x8@W8 + x8@Wr8 + xr8@W8        (drop xr8@Wr8, ~2^-8 relative)

DoubleRow pairs two 128-row contraction subtiles per PE instruction at half
the per-column cost, so the 3-term product costs 0.75x of one bf16 matmul
while keeping ~2^-8 effective precision (measured end-to-end rel err ~2e-3).

Weights are pre-scaled by 32 before fp8 quantization so their ~0.02-scale
entries sit in e4m3's normal range; the 1/32 descale folds into the mm1
activation scale and the per-token gate (g/32) on mm2 eviction.

Device layout keeps the contraction dim on SBUF partitions throughout:
  mm1: psum[h_blk(128), tok] += DR-pairs of {wfc8,wfcr8}[d,h_blk] x {x8,xr8}[d,tok]
  act: t = prelu(psum/32, 0.5) (fp16); af = t*t (fp16);
       a8 = fp8(af); ar8 = fp8(af - a8)
  mm2: psum[tok(128), d(512)] += DR-pairs of {a8,ar8}[h,tok] x {wp8,wpr8}[h,d]
  evict: out = psum * (g[token]/32)

Host scatters per-expert rows back (each token appears in exactly 2 expert
lists) and sums - identical math to the reference's dense masked combine.
"""

import numpy as np
import ml_dtypes
from contextlib import ExitStack

B, T, D, H, E = 4, 2048, 1024, 4096, 8
N = B * T
P = 128
CHUNK = 512
SW = 32.0  # weight pre-scale before fp8 quantization

E4NP = ml_dtypes.float8_e4m3  # == concourse mybir.dt.float8e4


_NC_CACHE = {}


def _build_nc(C):
    """Build + compile the per-core Bass program for capacity C tokens.

    C must be a multiple of 128. Tokens stream in chunks of 512 plus one
    optional tail chunk of C % 512. The program depends only on C, so it is
    cached: reusing the same nc object also lets bass2jax's jit cache skip
    the NEFF compile on repeat kernel() calls.
    """
    if C in _NC_CACHE:
        return _NC_CACHE[C]
    import concourse.bacc as bacc
    import concourse.tile as tile
    import concourse.mybir as mybir

    assert C % P == 0
    f8 = mybir.dt.float8e4
    f16 = mybir.dt.float16
    f32 = mybir.dt.float32
    AF = mybir.ActivationFunctionType
    DR = mybir.MatmulPerfMode.DoubleRow
    MUL = mybir.AluOpType.mult
    SUB = mybir.AluOpType.subtract

    nc = bacc.Bacc(None, target_bir_lowering=False, debug=False)
    x8T = nc.dram_tensor("x8T", [D, C], f8, kind="ExternalInput")
    xr8T = nc.dram_tensor("xr8T", [D, C], f8, kind="ExternalInput")
    wfc8T = nc.dram_tensor("wfc8T", [D, H], f8, kind="ExternalInput")
    wfcr8T = nc.dram_tensor("wfcr8T", [D, H], f8, kind="ExternalInput")
    wp8T = nc.dram_tensor("wp8T", [H, D], f8, kind="ExternalInput")
    wpr8T = nc.dram_tensor("wpr8T", [H, D], f8, kind="ExternalInput")
    g = nc.dram_tensor("g", [P, C // P], f32, kind="ExternalInput")
    out = nc.dram_tensor("outp", [C, D], f32, kind="ExternalOutput")

    x8T_v = x8T.ap().rearrange("(ko p) c -> p ko c", p=P)        # [128, 8, C]
    xr8T_v = xr8T.ap().rearrange("(ko p) c -> p ko c", p=P)
    wfc8_v = wfc8T.ap().rearrange("(ko p) h -> p ko h", p=P)     # [128, 8, H]
    wfcr8_v = wfcr8T.ap().rearrange("(ko p) h -> p ko h", p=P)
    wp8_v = wp8T.ap().rearrange("(ko p) d -> p ko d", p=P)       # [128, 32, D]
    wpr8_v = wpr8T.ap().rearrange("(ko p) d -> p ko d", p=P)
    out_v = out.ap().rearrange("(c p) d -> p c d", p=P)          # [128, C//128, D]

    KD = D // P          # 8  k-subtiles for mm1
    KH = H // P          # 32 k-subtiles for mm2 (and h-blocks of mm1 output)
    DN = D // CHUNK      # 2 output-column blocks

    chunks = [CHUNK] * (C // CHUNK)
    if C % CHUNK:
        chunks.append(C % CHUNK)
    # A short tail chunk (<256 tokens) stalls the PE on activation-chain
    # latency (mm1 h-blocks outpace the prelu/square/quantize pipeline);
    # rebalance the last two chunks to keep every chunk >= 256 tokens.
    if len(chunks) >= 2 and chunks[-1] < 2 * P:
        pair = chunks[-2] + chunks[-1]
        a = (pair // 2) // P * P
        chunks[-2:] = [a, pair - a]

    with tile.TileContext(nc) as tc:
        with ExitStack() as ctx:
            const = ctx.enter_context(tc.tile_pool(name="const", bufs=1))
            xpool = ctx.enter_context(tc.tile_pool(name="xp", bufs=2))
            apool = ctx.enter_context(tc.tile_pool(name="apool", bufs=1))
            tpool = ctx.enter_context(tc.tile_pool(name="tp", bufs=2))
            opool = ctx.enter_context(tc.tile_pool(name="op", bufs=4))
            ps1pool = ctx.enter_context(tc.tile_pool(name="ps1", bufs=4, space="PSUM"))
            ps2pool = ctx.enter_context(tc.tile_pool(name="ps2", bufs=3, space="PSUM"))
            warmpool = ctx.enter_context(tc.tile_pool(name="wm", bufs=1, space="PSUM"))

            # Startup-critical DMAs first. DGE descriptor generation is a
            # serialized per-path resource, so use few, large transfers and
            # spread them over all three DGE paths: SP (sync), Activation
            # (scalar) and SWDGE (gpsimd). mm1 h-block mh of chunk 0 reads
            # (in order) x8+wfc8 cols of mh, xr8, then wfcr8 cols of mh.
            x8_tiles = {}
            xr8_tiles = {}
            x8_tiles[0] = xpool.tile([P, KD, chunks[0]], f8, tag="x8", name="x8t0")
            xr8_tiles[0] = xpool.tile([P, KD, chunks[0]], f8, tag="xr8", name="xr8t0")
            wfc8_sb = const.tile([P, KD, H], f8)
            wfcr8_sb = const.tile([P, KD, H], f8)
            # The HWDGE descriptor-generation path (shared by SP+Activation)
            # and the DMA transfer pipe are both serialized resources, and
            # transfers run in generation order. Rules: (1) no DMAs on the
            # Activation engine — its in-order SEQ would stall the act chain
            # behind DMA generation; (2) the handful of immediately-needed
            # transfers go via the Pool-engine SWDGE (parallel generation);
            # (3) everything else goes on SP strictly in need order.
            nc.sync.dma_start(x8_tiles[0][:, 0:4, :], x8T_v[:, 0:4, 0:chunks[0]])
            nc.sync.dma_start(wfc8_sb[:, :, 0:P], wfc8_v[:, :, 0:P])
            nc.sync.dma_start(x8_tiles[0][:, 4:KD, :], x8T_v[:, 4:KD, 0:chunks[0]])
            nc.sync.dma_start(xr8_tiles[0][:, 0:4, :], xr8T_v[:, 0:4, 0:chunks[0]])
            nc.sync.dma_start(xr8_tiles[0][:, 4:KD, :], xr8T_v[:, 4:KD, 0:chunks[0]])
            nc.sync.dma_start(wfcr8_sb[:, :, 0:P], wfcr8_v[:, :, 0:P])
            # Rest of wfc8/wfcr8 in H-slices, finer early (tighter deadline),
            # coarser later: mm1 h-block mh only depends on the slices
            # covering its 128 cols (subregion deps).
            bounds = [P, 3 * P, 7 * P, 11 * P, 15 * P, 19 * P, 23 * P,
                      27 * P, H]
            for s0, s1 in zip(bounds[:-1], bounds[1:]):
                nc.sync.dma_start(
                    wfc8_sb[:, :, s0:s1], wfc8_v[:, :, s0:s1]
                )
                nc.sync.dma_start(
                    wfcr8_sb[:, :, s0:s1], wfcr8_v[:, :, s0:s1]
                )
            # Chunk-1 tokens next (needed when chunk-1 mm1 starts ~46us in),
            # then wproj for mm2 of chunk 0: dn=0 halves first (earlier
            # deadline); the dn=1 halves are emitted in the chunk-0 body.
            if len(chunks) > 1:
                S1 = chunks[1]
                x8_tiles[1] = xpool.tile([P, KD, S1], f8, tag="x8", name="x8t1")
                xr8_tiles[1] = xpool.tile([P, KD, S1], f8, tag="xr8", name="xr8t1")
                nc.sync.dma_start(x8_tiles[1][:], x8T_v[:, :, chunks[0]:chunks[0] + S1])
                nc.sync.dma_start(xr8_tiles[1][:], xr8T_v[:, :, chunks[0]:chunks[0] + S1])
            wp8_sb = const.tile([P, KH, D], f8)
            wpr8_sb = const.tile([P, KH, D], f8)
            g_sb = const.tile([P, C // P], f32)
            nc.sync.dma_start(wp8_sb[:, :, 0:CHUNK], wp8_v[:, :, 0:CHUNK])
            nc.sync.dma_start(wpr8_sb[:, :, 0:CHUNK], wpr8_v[:, :, 0:CHUNK])
            nc.gpsimd.dma_start(g_sb[:], g.ap())

            # PE warmup: the HAM clock-gate needs ~3.4us of sustained matmul
            # activity to grant the 2.4 GHz rate. The PE is idle waiting for
            # the first DMAs anyway, so burn that window on dummy matmuls
            # over a zeroed scratch tile (results never read).
            warm_sb = const.tile([P, P], f16)
            nc.vector.memset(warm_sb[:], 0.0)
            warm_ps = warmpool.tile([P, P], f32)
            for _ in range(38):
                nc.tensor.matmul(warm_ps[:], warm_sb[:], warm_sb[:],
                                 start=True, stop=True)

            tok0 = 0
            for c, S in enumerate(chunks):
                x8_t = x8_tiles[c]
                xr8_t = xr8_tiles[c]
                a8_t = apool.tile([P, KH, S], f8, tag="a8")
                ar8_t = apool.tile([P, KH, S], f8, tag="ar8")
                for mh in range(KH):
                    ps1 = ps1pool.tile([P, S], f32, tag="ps1")
                    mcols = slice(mh * P, (mh + 1) * P)
                    for k in range(0, KD, 2):
                        nc.tensor.matmul(
                            ps1[:], wfc8_sb[:, k:k + 2, mcols],
                            x8_t[:, k:k + 2, :],
                            start=(k == 0), stop=False, perf_mode=DR,
                        )
                    for k in range(0, KD, 2):
                        nc.tensor.matmul(
                            ps1[:], wfc8_sb[:, k:k + 2, mcols],
                            xr8_t[:, k:k + 2, :],
                            start=False, stop=False, perf_mode=DR,
                        )
                    for k in range(0, KD, 2):
                        nc.tensor.matmul(
                            ps1[:], wfcr8_sb[:, k:k + 2, mcols],
                            x8_t[:, k:k + 2, :],
                            start=False, stop=(k == KD - 2), perf_mode=DR,
                        )
                    # t = prelu(psum/32, 0.5); af = t*t; a8 = fp8(af);
                    # ar8 = fp8(af - a8)
                    t_t = tpool.tile([P, S], f16, tag="t")
                    nc.scalar.activation(t_t[:], ps1[:], AF.Prelu,
                                         alpha=0.5, scale=1.0 / SW)
                    af_t = tpool.tile([P, S], f16, tag="af")
                    nc.vector.tensor_tensor(af_t[:], t_t[:], t_t[:], MUL)
                    nc.scalar.activation(a8_t[:, mh, :], af_t[:], AF.Copy)
                    nc.vector.tensor_tensor(
                        ar8_t[:, mh, :], af_t[:], a8_t[:, mh, :], SUB
                    )
                # Prefetch next chunk's tokens now, ahead of this chunk's
                # out-DMAs in the DGE queues.
                if c + 1 < len(chunks) and c + 1 not in x8_tiles:
                    S1 = chunks[c + 1]
                    t1 = tok0 + S
                    x8_tiles[c + 1] = xpool.tile(
                        [P, KD, S1], f8, tag="x8", name=f"x8t{c + 1}"
                    )
                    xr8_tiles[c + 1] = xpool.tile(
                        [P, KD, S1], f8, tag="xr8", name=f"xr8t{c + 1}"
                    )
                    nc.sync.dma_start(x8_tiles[c + 1][:], x8T_v[:, :, t1:t1 + S1])
                    nc.sync.dma_start(xr8_tiles[c + 1][:], xr8T_v[:, :, t1:t1 + S1])
                if c == 0:
                    nc.sync.dma_start(wp8_sb[:, :, CHUNK:D], wp8_v[:, :, CHUNK:D])
                    nc.sync.dma_start(wpr8_sb[:, :, CHUNK:D], wpr8_v[:, :, CHUNK:D])
                for ti in range(S // P):
                    gcol = tok0 // P + ti
                    tcols = slice(ti * P, (ti + 1) * P)
                    for dn in range(DN):
                        dcols = slice(dn * CHUNK, (dn + 1) * CHUNK)
                        ps2 = ps2pool.tile([P, CHUNK], f32, tag="ps2")
                        for k in range(0, KH, 2):
                            nc.tensor.matmul(
                                ps2[:], a8_t[:, k:k + 2, tcols],
                                wp8_sb[:, k:k + 2, dcols],
                                start=(k == 0), stop=False, perf_mode=DR,
                            )
                        for k in range(0, KH, 2):
                            nc.tensor.matmul(
                                ps2[:], ar8_t[:, k:k + 2, tcols],
                                wp8_sb[:, k:k + 2, dcols],
                                start=False, stop=False, perf_mode=DR,
                            )
                        for k in range(0, KH, 2):
                            nc.tensor.matmul(
                                ps2[:], a8_t[:, k:k + 2, tcols],
                                wpr8_sb[:, k:k + 2, dcols],
                                start=False, stop=(k == KH - 2), perf_mode=DR,
                            )
                        last_tile = (c == len(chunks) - 1 and ti == S // P - 1
                                     and dn == DN - 1)
                        if not last_tile:
                            o_tile = opool.tile([P, CHUNK], f32, tag="ot")
                            # fused gate+descale: out = psum * (g[token]/32)
                            nc.scalar.activation(
                                o_tile[:], ps2[:], AF.Copy,
                                scale=g_sb[:, gcol:gcol + 1],
                            )
                            nc.sync.dma_start(
                                out_v[:, gcol, dcols], o_tile[:]
                            )
                        else:
                            # Final eviction in halves so the last DMA is
                            # small — shortens the end-of-kernel drain.
                            HC = CHUNK // 2
                            for hf in range(2):
                                o_half = opool.tile([P, HC], f32, tag="oth")
                                nc.scalar.activation(
                                    o_half[:], ps2[:, hf * HC:(hf + 1) * HC],
                                    AF.Copy, scale=g_sb[:, gcol:gcol + 1],
                                )
                                nc.sync.dma_start(
                                    out_v[:, gcol,
                                          dn * CHUNK + hf * HC:
                                          dn * CHUNK + (hf + 1) * HC],
                                    o_half[:],
                                )
                tok0 += S
    nc.compile()
    _NC_CACHE[C] = nc
    return nc


def _route(xf, Wg):
    """Exact top-2 gating in fp32, mirroring the reference math."""
    logits = xf @ Wg.T                                   # [N, E]
    top2 = np.argpartition(logits, E - 2, axis=1)[:, E - 2:]   # [N, 2] unordered
    vals = np.take_along_axis(logits, top2, axis=1)
    m = vals.max(axis=1, keepdims=True)
    ex = np.exp(vals - m)
    w = ex / ex.sum(axis=1, keepdims=True)               # [N, 2] softmax over top-2
    return top2, w


def _q8(v):
    """fp8(e4m3) base + residual split of a fp32 array."""
    base = v.astype(E4NP)
    resid = (v - base.astype(np.float32)).astype(E4NP)
    return base, resid


def run_moe(x, Wg, Wfc, Wproj, trace=False):
    from concourse import bass_utils

    xf = np.ascontiguousarray(x.reshape(-1, D), dtype=np.float32)
    top2, w = _route(xf, Wg.astype(np.float32))

    toks, gates = [], []
    for e in range(E):
        sel = np.nonzero((top2 == e).any(axis=1))[0]
        ge = (w[sel] * (top2[sel] == e)).sum(axis=1).astype(np.float32)
        toks.append(sel)
        gates.append(ge)

    maxc = max(len(t) for t in toks)
    C = max(P, ((maxc + P - 1) // P) * P)

    nc = _build_nc(C)

    x8_full, xr8_full = _q8(xf)
    in_maps = []
    for e in range(E):
        te = toks[e]
        x8T_e = np.zeros((D, C), E4NP)
        x8T_e[:, :len(te)] = x8_full[te].T
        xr8T_e = np.zeros((D, C), E4NP)
        xr8T_e[:, :len(te)] = xr8_full[te].T
        wfc8, wfcr8 = _q8(np.ascontiguousarray(Wfc[e].T, np.float32) * SW)
        wp8, wpr8 = _q8(np.ascontiguousarray(Wproj[e].T, np.float32) * SW)
        g_e = np.zeros((C,), np.float32)
        g_e[:len(te)] = gates[e] / SW
        g_mat = np.ascontiguousarray(g_e.reshape(C // P, P).T)
        in_maps.append({
            "x8T": x8T_e,
            "xr8T": xr8T_e,
            "wfc8T": wfc8,
            "wfcr8T": wfcr8,
            "wp8T": wp8,
            "wpr8T": wpr8,
            "g": g_mat,
        })

    # NTFF tracing is unavailable under this axon environment (no
    # antenv.axon_hooks); always run untraced.
    res = bass_utils.run_bass_kernel_spmd(
        nc, in_maps, core_ids=list(range(E)), trace=False
    )

    out = np.zeros((N, D), np.float32)
    for e in range(E):
        te = toks[e]
        out[te] += res.results[e]["outp"][:len(te)]
    return out.reshape(B, T, D), res


def kernel(x, Wg, Wfc, Wproj):
    out, _ = run_moe(np.asarray(x), np.asarray(Wg), np.asarray(Wfc), np.asarray(Wproj))
    return out


# revision 24
# speedup vs baseline: 1.1279x; 1.0082x over previous
"""MoE MLP (top-2 of 8 experts) Trainium2 kernel.

Strategy: expert-parallel across the 8 NeuronCores (host computes the tiny
top-2 gating exactly in fp32 and gathers each expert's tokens into a
capacity-padded buffer; core e runs expert e's two big matmuls).

The matmuls run as fp8(e4m3) DoubleRow matmuls with a 3-term residual
decomposition. Each operand is split into an fp8 base plus an fp8 residual
(quantization error of the base), and the product keeps the three large
cross-terms:

    x @ W  ~=  x8@W8 + x8@Wr8 + xr8@W8        (drop xr8@Wr8, ~2^-8 relative)

DoubleRow pairs two 128-row contraction subtiles per PE instruction at half
the per-column cost, so the 3-term product costs 0.75x of one bf16 matmul
while keeping ~2^-8 effective precision (measured end-to-end rel err ~2e-3).

Weights are pre-scaled by 32 before fp8 quantization so their ~0.02-scale
entries sit in e4m3's normal range; the 1/32 descale folds into the mm1
activation scale and the per-token gate (g/32) on mm2 eviction.

Device layout keeps the contraction dim on SBUF partitions throughout:
  mm1: psum[h_blk(128), tok] += DR-pairs of {wfc8,wfcr8}[d,h_blk] x {x8,xr8}[d,tok]
  act: t = prelu(psum/32, 0.5) (fp16); af = t*t (fp16);
       a8 = fp8(af); ar8 = fp8(af - a8)
  mm2: psum[tok(128), d(512)] += DR-pairs of {a8,ar8}[h,tok] x {wp8,wpr8}[h,d]
  evict: out = psum * (g[token]/32)

Host scatters per-expert rows back (each token appears in exactly 2 expert
lists) and sums - identical math to the reference's dense masked combine.
"""

import numpy as np
import ml_dtypes
from contextlib import ExitStack

B, T, D, H, E = 4, 2048, 1024, 4096, 8
N = B * T
P = 128
CHUNK = 512
SW = 32.0  # weight pre-scale before fp8 quantization

E4NP = ml_dtypes.float8_e4m3  # == concourse mybir.dt.float8e4

# Tuning knobs (resolved at _build_nc time; the cache key includes them).
TUNE = {
    "tail_mode": "rebalance",  # "rebalance" | "plain"
    "tpool_bufs": 2,
    "warmup": 38,
    "warm_memset_engine": "vector",  # "vector" | "gpsimd"
}

_NC_CACHE = {}


def _build_nc(C):
    """Build + compile the per-core Bass program for capacity C tokens.

    C must be a multiple of 128. Tokens stream in chunks of 512 plus one
    optional tail chunk of C % 512. The program depends only on C, so it is
    cached: reusing the same nc object also lets bass2jax's jit cache skip
    the NEFF compile on repeat kernel() calls.
    """
    key = (C, tuple(sorted(TUNE.items())))
    if key in _NC_CACHE:
        return _NC_CACHE[key]
    import concourse.bacc as bacc
    import concourse.tile as tile
    import concourse.mybir as mybir

    assert C % P == 0
    f8 = mybir.dt.float8e4
    f16 = mybir.dt.float16
    f32 = mybir.dt.float32
    AF = mybir.ActivationFunctionType
    DR = mybir.MatmulPerfMode.DoubleRow
    MUL = mybir.AluOpType.mult
    SUB = mybir.AluOpType.subtract

    nc = bacc.Bacc(None, target_bir_lowering=False, debug=False)
    x8T = nc.dram_tensor("x8T", [D, C], f8, kind="ExternalInput")
    xr8T = nc.dram_tensor("xr8T", [D, C], f8, kind="ExternalInput")
    wfc8T = nc.dram_tensor("wfc8T", [D, H], f8, kind="ExternalInput")
    wfcr8T = nc.dram_tensor("wfcr8T", [D, H], f8, kind="ExternalInput")
    wp8T = nc.dram_tensor("wp8T", [H, D], f8, kind="ExternalInput")
    wpr8T = nc.dram_tensor("wpr8T", [H, D], f8, kind="ExternalInput")
    g = nc.dram_tensor("g", [P, C // P], f32, kind="ExternalInput")
    out = nc.dram_tensor("outp", [C, D], f32, kind="ExternalOutput")

    x8T_v = x8T.ap().rearrange("(ko p) c -> p ko c", p=P)        # [128, 8, C]
    xr8T_v = xr8T.ap().rearrange("(ko p) c -> p ko c", p=P)
    wfc8_v = wfc8T.ap().rearrange("(ko p) h -> p ko h", p=P)     # [128, 8, H]
    wfcr8_v = wfcr8T.ap().rearrange("(ko p) h -> p ko h", p=P)
    wp8_v = wp8T.ap().rearrange("(ko p) d -> p ko d", p=P)       # [128, 32, D]
    wpr8_v = wpr8T.ap().rearrange("(ko p) d -> p ko d", p=P)
    out_v = out.ap().rearrange("(c p) d -> p c d", p=P)          # [128, C//128, D]

    KD = D // P          # 8  k-subtiles for mm1
    KH = H // P          # 32 k-subtiles for mm2 (and h-blocks of mm1 output)
    DN = D // CHUNK      # 2 output-column blocks

    chunks = [CHUNK] * (C // CHUNK)
    if C % CHUNK:
        chunks.append(C % CHUNK)
    # Short chunks stall the PE: the per-block activation chain has a fixed
    # ~160ns/op overhead, so below ~384 tokens ScalarE/DVE can't keep pace
    # with mm1 h-blocks. Rebalance by borrowing 128-token slices from full
    # chunks until every chunk is >= 384 tokens.
    if TUNE["tail_mode"] == "rebalance":
        while chunks[-1] < 3 * P:
            idx = next((i for i in range(len(chunks) - 1, -1, -1)
                        if chunks[i] >= CHUNK), None)
            if idx is None:
                break
            chunks[idx] -= P
            chunks[-1] += P

    with tile.TileContext(nc) as tc:
        with ExitStack() as ctx:
            const = ctx.enter_context(tc.tile_pool(name="const", bufs=1))
            xpool = ctx.enter_context(tc.tile_pool(name="xp", bufs=2))
            apool = ctx.enter_context(tc.tile_pool(name="apool", bufs=1))
            tpool = ctx.enter_context(tc.tile_pool(name="tp", bufs=TUNE["tpool_bufs"]))
            opool = ctx.enter_context(tc.tile_pool(name="op", bufs=4))
            ps1pool = ctx.enter_context(tc.tile_pool(name="ps1", bufs=4, space="PSUM"))
            ps2pool = ctx.enter_context(tc.tile_pool(name="ps2", bufs=3, space="PSUM"))
            warmpool = ctx.enter_context(tc.tile_pool(name="wm", bufs=1, space="PSUM"))

            # Startup-critical DMAs first. DGE descriptor generation is a
            # serialized per-path resource, so use few, large transfers and
            # spread them over all three DGE paths: SP (sync), Activation
            # (scalar) and SWDGE (gpsimd). mm1 h-block mh of chunk 0 reads
            # (in order) x8+wfc8 cols of mh, xr8, then wfcr8 cols of mh.
            x8_tiles = {}
            xr8_tiles = {}
            x8_tiles[0] = xpool.tile([P, KD, chunks[0]], f8, tag="x8", name="x8t0")
            xr8_tiles[0] = xpool.tile([P, KD, chunks[0]], f8, tag="xr8", name="xr8t0")
            wfc8_sb = const.tile([P, KD, H], f8)
            wfcr8_sb = const.tile([P, KD, H], f8)
            # The HWDGE descriptor-generation path (shared by SP+Activation)
            # and the DMA transfer pipe are both serialized resources, and
            # transfers run in generation order. Rules: (1) no DMAs on the
            # Activation engine — its in-order SEQ would stall the act chain
            # behind DMA generation; (2) the handful of immediately-needed
            # transfers go via the Pool-engine SWDGE (parallel generation);
            # (3) everything else goes on SP strictly in need order.
            nc.sync.dma_start(x8_tiles[0][:, 0:4, :], x8T_v[:, 0:4, 0:chunks[0]])
            nc.sync.dma_start(wfc8_sb[:, :, 0:P], wfc8_v[:, :, 0:P])
            nc.sync.dma_start(x8_tiles[0][:, 4:KD, :], x8T_v[:, 4:KD, 0:chunks[0]])
            nc.sync.dma_start(xr8_tiles[0][:, 0:4, :], xr8T_v[:, 0:4, 0:chunks[0]])
            nc.sync.dma_start(xr8_tiles[0][:, 4:KD, :], xr8T_v[:, 4:KD, 0:chunks[0]])
            nc.sync.dma_start(wfcr8_sb[:, :, 0:P], wfcr8_v[:, :, 0:P])
            # Rest of wfc8/wfcr8 in H-slices, finer early (tighter deadline),
            # coarser later: mm1 h-block mh only depends on the slices
            # covering its 128 cols (subregion deps).
            bounds = [P, 3 * P, 7 * P, 11 * P, 15 * P, 19 * P, 23 * P,
                      27 * P, H]
            for s0, s1 in zip(bounds[:-1], bounds[1:]):
                nc.sync.dma_start(
                    wfc8_sb[:, :, s0:s1], wfc8_v[:, :, s0:s1]
                )
                nc.sync.dma_start(
                    wfcr8_sb[:, :, s0:s1], wfcr8_v[:, :, s0:s1]
                )
            # Chunk-1 tokens next (needed when chunk-1 mm1 starts ~46us in),
            # then wproj for mm2 of chunk 0: dn=0 halves first (earlier
            # deadline); the dn=1 halves are emitted in the chunk-0 body.
            if len(chunks) > 1:
                S1 = chunks[1]
                x8_tiles[1] = xpool.tile([P, KD, S1], f8, tag="x8", name="x8t1")
                xr8_tiles[1] = xpool.tile([P, KD, S1], f8, tag="xr8", name="xr8t1")
                nc.sync.dma_start(x8_tiles[1][:], x8T_v[:, :, chunks[0]:chunks[0] + S1])
                nc.sync.dma_start(xr8_tiles[1][:], xr8T_v[:, :, chunks[0]:chunks[0] + S1])
            wp8_sb = const.tile([P, KH, D], f8)
            wpr8_sb = const.tile([P, KH, D], f8)
            g_sb = const.tile([P, C // P], f32)
            nc.sync.dma_start(wp8_sb[:, :, 0:CHUNK], wp8_v[:, :, 0:CHUNK])
            nc.sync.dma_start(wpr8_sb[:, :, 0:CHUNK], wpr8_v[:, :, 0:CHUNK])
            nc.gpsimd.dma_start(g_sb[:], g.ap())

            # PE warmup: the HAM clock-gate needs ~3.4us of sustained matmul
            # activity to grant the 2.4 GHz rate. The PE is idle waiting for
            # the first DMAs anyway, so burn that window on dummy matmuls
            # over a zeroed scratch tile (results never read).
            warm_sb = const.tile([P, P], f16)
            if TUNE["warm_memset_engine"] == "gpsimd":
                nc.gpsimd.memset(warm_sb[:], 0.0)
            else:
                nc.vector.memset(warm_sb[:], 0.0)
            warm_ps = warmpool.tile([P, P], f32)
            for _ in range(TUNE["warmup"]):
                nc.tensor.matmul(warm_ps[:], warm_sb[:], warm_sb[:],
                                 start=True, stop=True)

            tok0 = 0
            for c, S in enumerate(chunks):
                x8_t = x8_tiles[c]
                xr8_t = xr8_tiles[c]
                a8_t = apool.tile([P, KH, S], f8, tag="a8")
                ar8_t = apool.tile([P, KH, S], f8, tag="ar8")
                for mh in range(KH):
                    ps1 = ps1pool.tile([P, S], f32, tag="ps1")
                    mcols = slice(mh * P, (mh + 1) * P)
                    for k in range(0, KD, 2):
                        nc.tensor.matmul(
                            ps1[:], wfc8_sb[:, k:k + 2, mcols],
                            x8_t[:, k:k + 2, :],
                            start=(k == 0), stop=False, perf_mode=DR,
                        )
                    for k in range(0, KD, 2):
                        nc.tensor.matmul(
                            ps1[:], wfc8_sb[:, k:k + 2, mcols],
                            xr8_t[:, k:k + 2, :],
                            start=False, stop=False, perf_mode=DR,
                        )
                    for k in range(0, KD, 2):
                        nc.tensor.matmul(
                            ps1[:], wfcr8_sb[:, k:k + 2, mcols],
                            x8_t[:, k:k + 2, :],
                            start=False, stop=(k == KD - 2), perf_mode=DR,
                        )
                    # t = prelu(psum/32, 0.5); af = t*t; a8 = fp8(af);
                    # ar8 = fp8(af - a8)
                    t_t = tpool.tile([P, S], f16, tag="t")
                    nc.scalar.activation(t_t[:], ps1[:], AF.Prelu,
                                         alpha=0.5, scale=1.0 / SW)
                    af_t = tpool.tile([P, S], f16, tag="af")
                    nc.vector.tensor_tensor(af_t[:], t_t[:], t_t[:], MUL)
                    nc.vector.tensor_copy(a8_t[:, mh, :], af_t[:])
                    nc.vector.tensor_tensor(
                        ar8_t[:, mh, :], af_t[:], a8_t[:, mh, :], SUB
                    )
                # Prefetch next chunk's tokens now, ahead of this chunk's
                # out-DMAs in the DGE queues.
                if c + 1 < len(chunks) and c + 1 not in x8_tiles:
                    S1 = chunks[c + 1]
                    t1 = tok0 + S
                    x8_tiles[c + 1] = xpool.tile(
                        [P, KD, S1], f8, tag="x8", name=f"x8t{c + 1}"
                    )
                    xr8_tiles[c + 1] = xpool.tile(
                        [P, KD, S1], f8, tag="xr8", name=f"xr8t{c + 1}"
                    )
                    nc.sync.dma_start(x8_tiles[c + 1][:], x8T_v[:, :, t1:t1 + S1])
                    nc.sync.dma_start(xr8_tiles[c + 1][:], xr8T_v[:, :, t1:t1 + S1])
                if c == 0:
                    nc.sync.dma_start(wp8_sb[:, :, CHUNK:D], wp8_v[:, :, CHUNK:D])
                    nc.sync.dma_start(wpr8_sb[:, :, CHUNK:D], wpr8_v[:, :, CHUNK:D])
                for ti in range(S // P):
                    gcol = tok0 // P + ti
                    tcols = slice(ti * P, (ti + 1) * P)
                    for dn in range(DN):
                        dcols = slice(dn * CHUNK, (dn + 1) * CHUNK)
                        ps2 = ps2pool.tile([P, CHUNK], f32, tag="ps2")
                        for k in range(0, KH, 2):
                            nc.tensor.matmul(
                                ps2[:], a8_t[:, k:k + 2, tcols],
                                wp8_sb[:, k:k + 2, dcols],
                                start=(k == 0), stop=False, perf_mode=DR,
                            )
                        for k in range(0, KH, 2):
                            nc.tensor.matmul(
                                ps2[:], ar8_t[:, k:k + 2, tcols],
                                wp8_sb[:, k:k + 2, dcols],
                                start=False, stop=False, perf_mode=DR,
                            )
                        for k in range(0, KH, 2):
                            nc.tensor.matmul(
                                ps2[:], a8_t[:, k:k + 2, tcols],
                                wpr8_sb[:, k:k + 2, dcols],
                                start=False, stop=(k == KH - 2), perf_mode=DR,
                            )
                        last_tile = (c == len(chunks) - 1 and ti == S // P - 1
                                     and dn == DN - 1)
                        if not last_tile:
                            o_tile = opool.tile([P, CHUNK], f32, tag="ot")
                            # fused gate+descale: out = psum * (g[token]/32)
                            nc.scalar.activation(
                                o_tile[:], ps2[:], AF.Copy,
                                scale=g_sb[:, gcol:gcol + 1],
                            )
                            nc.sync.dma_start(
                                out_v[:, gcol, dcols], o_tile[:]
                            )
                        else:
                            # Final eviction in halves so the last DMA is
                            # small — shortens the end-of-kernel drain.
                            HC = CHUNK // 2
                            for hf in range(2):
                                o_half = opool.tile([P, HC], f32, tag="oth")
                                nc.scalar.activation(
                                    o_half[:], ps2[:, hf * HC:(hf + 1) * HC],
                                    AF.Copy, scale=g_sb[:, gcol:gcol + 1],
                                )
                                nc.sync.dma_start(
                                    out_v[:, gcol,
                                          dn * CHUNK + hf * HC:
                                          dn * CHUNK + (hf + 1) * HC],
                                    o_half[:],
                                )
                tok0 += S
    nc.compile()
    _NC_CACHE[key] = nc
    return nc


def _route(xf, Wg):
    """Exact top-2 gating in fp32, mirroring the reference math."""
    logits = xf @ Wg.T                                   # [N, E]
    top2 = np.argpartition(logits, E - 2, axis=1)[:, E - 2:]   # [N, 2] unordered
    vals = np.take_along_axis(logits, top2, axis=1)
    m = vals.max(axis=1, keepdims=True)
    ex = np.exp(vals - m)
    w = ex / ex.sum(axis=1, keepdims=True)               # [N, 2] softmax over top-2
    return top2, w


def _q8(v):
    """fp8(e4m3) base + residual split of a fp32 array."""
    base = v.astype(E4NP)
    resid = (v - base.astype(np.float32)).astype(E4NP)
    return base, resid


def run_moe(x, Wg, Wfc, Wproj, trace=False):
    from concourse import bass_utils

    xf = np.ascontiguousarray(x.reshape(-1, D), dtype=np.float32)
    top2, w = _route(xf, Wg.astype(np.float32))

    toks, gates = [], []
    for e in range(E):
        sel = np.nonzero((top2 == e).any(axis=1))[0]
        ge = (w[sel] * (top2[sel] == e)).sum(axis=1).astype(np.float32)
        toks.append(sel)
        gates.append(ge)

    maxc = max(len(t) for t in toks)
    C = max(P, ((maxc + P - 1) // P) * P)

    nc = _build_nc(C)

    x8_full, xr8_full = _q8(xf)
    in_maps = []
    for e in range(E):
        te = toks[e]
        x8T_e = np.zeros((D, C), E4NP)
        x8T_e[:, :len(te)] = x8_full[te].T
        xr8T_e = np.zeros((D, C), E4NP)
        xr8T_e[:, :len(te)] = xr8_full[te].T
        wfc8, wfcr8 = _q8(np.ascontiguousarray(Wfc[e].T, np.float32) * SW)
        wp8, wpr8 = _q8(np.ascontiguousarray(Wproj[e].T, np.float32) * SW)
        g_e = np.zeros((C,), np.float32)
        g_e[:len(te)] = gates[e] / SW
        g_mat = np.ascontiguousarray(g_e.reshape(C // P, P).T)
        in_maps.append({
            "x8T": x8T_e,
            "xr8T": xr8T_e,
            "wfc8T": wfc8,
            "wfcr8T": wfcr8,
            "wp8T": wp8,
            "wpr8T": wpr8,
            "g": g_mat,
        })

    # NTFF tracing is unavailable under this axon environment (no
    # antenv.axon_hooks); always run untraced.
    res = bass_utils.run_bass_kernel_spmd(
        nc, in_maps, core_ids=list(range(E)), trace=False
    )

    out = np.zeros((N, D), np.float32)
    for e in range(E):
        te = toks[e]
        out[te] += res.results[e]["outp"][:len(te)]
    return out.reshape(B, T, D), res


def kernel(x, Wg, Wfc, Wproj):
    out, _ = run_moe(np.asarray(x), np.asarray(Wg), np.asarray(Wfc), np.asarray(Wproj))
    return out
